# revision 1
# baseline (speedup 1.0000x reference)
"""Trainium2 Bass kernel for nn_AFiReLoss (SwAV-style sinkhorn CE + recon MSE).

Distribution: data-parallel over batch B=64 across 8 NeuronCores (8 per core).
The teacher batch-sum is ReduceScattered so each core owns a K-shard of the
prototype; sinkhorn runs K-sharded (col sums all-reduced, 196 floats/iter);
the normalized teacher targets are AllGathered for the local CE pass.

Math notes:
  per_patch[b,l] = lse(student[b,l,:]/T) * rowsum_t[l] - (1/T) * <student, t>
  and after the final col-normalization of sinkhorn, rowsum_t[l] == 1 exactly.
  The initial Q/sum(Q) normalization cancels in the first row-normalization
  and is skipped (values stay well inside f32 range).
"""

import numpy as np
import ml_dtypes

import concourse.bass as bass
import concourse.mybir as mybir
from concourse import tile, bacc
from concourse.bass_utils import run_bass_kernel_spmd
from concourse.masks import make_identity

F32 = mybir.dt.float32
BF16 = mybir.dt.bfloat16
AX = mybir.AxisListType
ALU = mybir.AluOpType
AF = mybir.ActivationFunctionType

P = 128              # SBUF partitions
N_CORES = 8
STUDENT_TEMP = 0.1
PROTO_MOMENTUM = 0.75
SK_EPS = 0.05
SK_ITERS = 3
LSE_SHIFT = 25.0   # global exp shift; added back in finalize()


def _ceil_div(a, b):
    return (a + b - 1) // b


def _tree_reduce_sum(nc, t_ap_fn, width, out_ap):
    """Free-axis sum via in-place halving adds (bf16 TT runs 2x; reduce is
    1x-only), then one short reduce_sum. t_ap_fn(lo, hi) -> AP slice."""
    w = width
    while w >= 1024 and w % 2 == 0:
        h = w // 2
        nc.vector.tensor_add(t_ap_fn(0, h), t_ap_fn(0, h), t_ap_fn(h, w))
        w = h
    nc.vector.reduce_sum(out_ap, t_ap_fn(0, w), axis=AX.X)


def build_nc(B_loc=8, L=196, K=8192, C=N_CORES, R=9408, dummy_out=True,
             stop_after="full"):
    """Build the per-core SPMD graph. R = recon elements per partition.

    stop_after: "rs" | "sinkhorn" | "full" — debugging aid that truncates
    the graph after the named phase (outputs are then meaningless).
    """
    KSH = K // C
    n_lt = _ceil_div(L, P)
    lt_sizes = [min(P, L - i * P) for i in range(n_lt)]
    n_t = B_loc * n_lt               # CE tiles per core

    nc = bacc.Bacc("TRN2", target_bir_lowering=False, debug=False,
                   num_devices=C)

    student = nc.declare_dram_parameter("student", [B_loc, L, K], BF16, isOutput=False)
    teacher = nc.declare_dram_parameter("teacher", [B_loc, L, K], BF16, isOutput=False)
    recon = nc.declare_dram_parameter("recon", [P, R], BF16, isOutput=False)
    label = nc.declare_dram_parameter("label", [P, R], BF16, isOutput=False)
    proto = nc.declare_dram_parameter("proto", [L, KSH], F32, isOutput=False)
    maskp = nc.declare_dram_parameter("maskp", [P, n_t], F32, isOutput=False)
    cfg = nc.declare_dram_parameter("cfg", [P, 1], F32, isOutput=False)
    out_ext = nc.declare_dram_parameter("out", [2 * P, 1], F32, isOutput=True)

    groups = [list(range(C))]

    with tile.TileContext(nc) as tc:
        with (
            tc.tile_pool(name="dram", bufs=1, space="DRAM") as dram,
            tc.tile_pool(name="consts", bufs=1) as consts,
            tc.tile_pool(name="small", bufs=2) as small,
            tc.tile_pool(name="spool", bufs=2) as spool,
            tc.tile_pool(name="scrp", bufs=1) as scrp,
            tc.tile_pool(name="tpool", bufs=2) as tpool,
        ):
            # ---- DRAM scratch (dep-tracked via DRAM pool) ----
            bsum_tiled = dram.tile([C, L, KSH], F32, tag="bsum_tiled")
            bsum_shard = dram.tile([L, KSH], F32, tag="bsum_shard")
            t_shard = dram.tile([L, KSH], BF16, tag="t_shard")
            t_all = dram.tile([C, L, KSH], BF16, tag="t_all")
            col_io = [
                (dram.tile([L, 1], F32, tag=f"col_in{i}", name=f"col_in{i}"),
                 dram.tile([L, 1], F32, tag=f"col_out{i}", name=f"col_out{i}"))
                for i in range(SK_ITERS)
            ]

            # ---- constants ----
            idbf = consts.tile([P, P], BF16, tag="idbf")
            make_identity(nc, idbf[:, :])
            idf32 = consts.tile([P, P], F32, tag="idf32")
            make_identity(nc, idf32[:, :])
            ones1 = consts.tile([1, P], F32, tag="ones1")
            nc.gpsimd.memset(ones1[:, :], 1.0)
            cL = consts.tile([P, 1], F32, tag="cL")
            nc.gpsimd.memset(cL[:, :], float(L))
            cfg_sb = consts.tile([P, 1], F32, tag="cfg_sb")
            nc.sync.dma_start(cfg_sb[:, :], cfg[:, :])
            mask_sb = consts.tile([P, n_t], F32, tag="mask_sb")
            nc.sync.dma_start(mask_sb[:, :], maskp[:, :])

            res = consts.tile([P, n_t], F32, tag="res")
            nc.gpsimd.memset(res[:, :], 0.0)
            nshift = consts.tile([P, 1], F32, tag="nshift")
            nc.gpsimd.memset(nshift[:, :], -LSE_SHIFT)
            # dead-store sinks for fused-reduce ops (free-stride-0 writes)
            dummy_bf = consts.tile([P, 1], BF16, tag="dummy_bf")

            # =========================================================
            # Phase 1: teacher batch-sum via PE identity-matmul accum
            # =========================================================
            KH = 2048 if K >= 2048 else K      # K columns per PSUM round
            n_rounds = _ceil_div(K, KH)
            assert KH % KSH == 0 or KSH % KH == 0
            with (
                tc.tile_pool(name="teach", bufs=2) as teach,
                tc.tile_pool(name="bsp", bufs=4, space="PSUM") as bsp,
                tc.tile_pool(name="bsev", bufs=2) as bsev,
            ):
                for lt, nl in enumerate(lt_sizes):
                    l0 = lt * P
                    for r in range(n_rounds):
                        k0 = r * KH
                        kw = min(KH, K - k0)
                        n_ch = _ceil_div(kw, 512)
                        psums = [bsp.tile([P, 512], F32, tag="bs", name=f"bs{r}_{i}")
                                 for i in range(n_ch)]
                        for b in range(B_loc):
                            tt = teach.tile([P, KH], BF16, tag="tt")
                            nc.sync.dma_start(
                                tt[:nl, :kw], teacher[b, l0:l0 + nl, k0:k0 + kw])
                            for ch in range(n_ch):
                                c0 = ch * 512
                                cw = min(512, kw - c0)
                                nc.tensor.matmul(
                                    psums[ch][:nl, :cw],
                                    idbf[:nl, :nl],
                                    tt[:nl, c0:c0 + cw],
                                    start=(b == 0), stop=(b == B_loc - 1))
                        stage = bsev.tile([P, KH], F32, tag="bsum_stage",
                                          name=f"stage{lt}_{r}")
                        for ch in range(n_ch):
                            c0 = ch * 512
                            cw = min(512, kw - c0)
                            nc.scalar.copy(stage[:nl, c0:c0 + cw],
                                           psums[ch][:nl, :cw])
                        # write k-tiled layout: RS chunk c = core c's K-shard
                        cs0 = k0 // KSH
                        ncs = max(1, kw // KSH)
                        nc.sync.dma_start(
                            bsum_tiled[cs0:cs0 + ncs, l0:l0 + nl,
                                       (k0 % KSH):(k0 % KSH) + min(kw, KSH)]
                            .rearrange("c l k -> l c k"),
                            stage[:nl, :kw].rearrange("l (c k) -> l c k", c=ncs))

            # =========================================================
            # Phase 1b: recon MSE partial on DVE
            # =========================================================
            RCH = 2352 if R > 2352 else R
            n_rch = _ceil_div(R, RCH)
            with tc.tile_pool(name="rec", bufs=1) as rec:
                rsq_prev = None
                for rc in range(n_rch):
                    r0 = rc * RCH
                    rw = min(RCH, R - r0)
                    r_sb = rec.tile([P, RCH], BF16, tag="r_sb", name=f"r_sb{rc}")
                    l_sb = rec.tile([P, RCH], BF16, tag="l_sb", name=f"l_sb{rc}")
                    d_sb = rec.tile([P, RCH], BF16, tag="d_sb", name=f"d_sb{rc}")
                    nc.sync.dma_start(r_sb[:, :rw], recon[:, r0:r0 + rw])
                    nc.sync.dma_start(l_sb[:, :rw], label[:, r0:r0 + rw])
                    rsq = small.tile([P, 1], F32, tag="rsq", name=f"rsq{rc}")
                    nc.vector.tensor_sub(d_sb[:, :rw], r_sb[:, :rw], l_sb[:, :rw])
                    nc.vector.tensor_mul(d_sb[:, :rw], d_sb[:, :rw], d_sb[:, :rw])
                    nc.vector.reduce_sum(rsq[:, 0:1], d_sb[:, :rw], axis=AX.X)
                    if rsq_prev is not None:
                        rsq2 = small.tile([P, 1], F32, tag="rsqs",
                                          name=f"rsqs{rc}")
                        nc.vector.tensor_add(rsq2[:, 0:1], rsq[:, 0:1],
                                             rsq_prev[:, 0:1])
                        rsq = rsq2
                    rsq_prev = rsq

            # =========================================================
            # Phase 2: ReduceScatter the batch-sum -> own K-shard
            # =========================================================
            nc.gpsimd.collective_compute(
                "ReduceScatter", ALU.add, replica_groups=groups,
                ins=[bsum_tiled.opt()], outs=[bsum_shard.opt()])

            if stop_after != "rs":
                # =========================================================
                # Phase 3: K-sharded sinkhorn on [L, KSH] f32
                # =========================================================
                n_kch = _ceil_div(KSH, P)          # 128-wide chunks of the shard
                with (
                    tc.tile_pool(name="skp", bufs=1) as skp,
                    tc.tile_pool(name="skpp", bufs=2, space="PSUM") as skpp,
                ):
                    Q = []
                    for lt, nl in enumerate(lt_sizes):
                        l0 = lt * P
                        pr = skp.tile([P, KSH], F32, tag=f"pr{lt}")
                        sh = skp.tile([P, KSH], F32, tag=f"sh{lt}")
                        nc.sync.dma_start(pr[:nl, :], proto[l0:l0 + nl, :])
                        nc.sync.dma_start(sh[:nl, :], bsum_shard[l0:l0 + nl, :])
                        q = skp.tile([P, KSH], F32, tag=f"q{lt}")
                        # q = exp(sh * cfg + pr)   (cfg = (1-m)/(64*eps) per-partition)
                        nc.vector.scalar_tensor_tensor(
                            q[:nl, :], in0=sh[:nl, :], scalar=cfg_sb[:nl, 0:1],
                            in1=pr[:nl, :], op0=ALU.mult, op1=ALU.add)
                        nc.scalar.activation(q[:nl, :], q[:nl, :], AF.Exp)
                        Q.append(q)

                    rb = skp.tile([P, KSH], F32, tag="rb")
                    for it in range(SK_ITERS):
                        # --- row step: r[k] = 1/(L * rowsum[k]), rowsum over l ---
                        ps_r = skpp.tile([P, n_kch], F32, tag="ps_r")
                        for ch in range(n_kch):
                            c0 = ch * P
                            cw = min(P, KSH - c0)
                            for lt, nl in enumerate(lt_sizes):
                                nc.tensor.matmul(
                                    ps_r[:cw, ch:ch + 1],
                                    Q[lt][:nl, c0:c0 + cw],
                                    cL[:nl, 0:1],
                                    start=(lt == 0), stop=(lt == n_lt - 1))
                        rmax = min(P, KSH)
                        rrec = small.tile([P, n_kch], F32, tag="rrec")
                        if rmax < P:
                            nc.vector.memset(rrec[:, :], 1.0)
                        nc.vector.reciprocal(rrec[:rmax, :], ps_r[:rmax, :])
                        # per chunk: transpose rowrecip column to a partition-0
                        # row, then PE-broadcast it across all 128 partitions
                        # (matmul operands must start at partition 0/32/64).
                        ps_b = skpp.tile([P, KSH], F32, tag="ps_b")
                        for ch in range(n_kch):
                            c0 = ch * P
                            cw = min(P, KSH - c0)
                            ps_t = skpp.tile([1, P], F32, tag="ps_t",
                                             name=f"ps_t{it}_{ch}")
                            nc.tensor.transpose(ps_t[0:1, :], rrec[:, ch:ch + 1],
                                                idf32[:, :])
                            r8row = small.tile([1, P], F32, tag="r8row",
                                               name=f"r8row{it}_{ch}")
                            nc.scalar.copy(r8row[0:1, :], ps_t[0:1, :])
                            nc.tensor.matmul(
                                ps_b[:, c0:c0 + cw],
                                ones1[0:1, :],
                                r8row[0:1, :cw],
                                start=True, stop=True)
                        nc.scalar.copy(rb[:, :], ps_b[:, :])
                        for lt, nl in enumerate(lt_sizes):
                            nc.vector.tensor_mul(Q[lt][:nl, :], Q[lt][:nl, :], rb[:nl, :])

                        # --- col step: c[l] = 1/(K * colsum[l]), colsum all-reduced ---
                        col_in, col_out = col_io[it]
                        colp = small.tile([P, 1], F32, tag="colp")
                        for lt, nl in enumerate(lt_sizes):
                            l0 = lt * P
                            nc.vector.reduce_sum(colp[:nl, 0:1], Q[lt][:nl, :], axis=AX.X)
                            nc.sync.dma_start(col_in[l0:l0 + nl, 0:1], colp[:nl, 0:1])
                        nc.gpsimd.collective_compute(
                            "AllReduce", ALU.add, replica_groups=groups,
                            ins=[col_in.opt()], outs=[col_out.opt()])
                        for lt, nl in enumerate(lt_sizes):
                            l0 = lt * P
                            csb = small.tile([P, 1], F32, tag="csb")
                            nc.sync.dma_start(csb[:nl, 0:1], col_out[l0:l0 + nl, 0:1])
                            nc.vector.tensor_scalar_mul(csb[:nl, 0:1], csb[:nl, 0:1],
                                                        float(K))
                            crec = small.tile([P, 1], F32, tag="crec")
                            nc.vector.reciprocal(crec[:nl, 0:1], csb[:nl, 0:1])
                            nc.vector.tensor_scalar_mul(Q[lt][:nl, :], Q[lt][:nl, :],
                                                        crec[:nl, 0:1])

                    # final targets: t = Q * K, cast to bf16, gather all shards
                    for lt, nl in enumerate(lt_sizes):
                        l0 = lt * P
                        tb = small.tile([P, KSH], BF16, tag="tb")
                        nc.vector.tensor_scalar_mul(tb[:nl, :], Q[lt][:nl, :], float(K))
                        nc.sync.dma_start(t_shard[l0:l0 + nl, :], tb[:nl, :])
                    nc.gpsimd.collective_compute(
                        "AllGather", ALU.bypass, replica_groups=groups,
                        ins=[t_shard.opt()], outs=[t_all.opt()])

                if stop_after == "full":
                    # =========================================================
                    # Phase 4: masked CE over student tiles
                    # =========================================================
                    t_sb = []
                    for lt, nl in enumerate(lt_sizes):
                        l0 = lt * P
                        ts = tpool.tile([P, K], BF16, tag="t_sb")
                        nc.sync.dma_start(
                            ts[:nl, :].rearrange("l (c k) -> l c k", c=C),
                            t_all[:, l0:l0 + nl, :].rearrange("c l k -> l c k"))
                        t_sb.append(ts)

                    inv_t = 1.0 / STUDENT_TEMP
                    for b in range(B_loc):
                        for lt, nl in enumerate(lt_sizes):
                            j = b * n_lt + lt
                            l0 = lt * P
                            s = spool.tile([P, K], BF16, tag="s")
                            nc.sync.dma_start(s[:nl, :], student[b, l0:l0 + nl, :])

                            # exp((x/T) - SHIFT): keeps Z inside ScalarE Ln's domain
                            # (Ln input must be < 2^64; raw Z can reach ~1e23).
                            z = small.tile([P, 1], F32, tag="z")
                            escr = scrp.tile([P, K], BF16, tag="escr",
                                             name=f"escr{j}", bufs=2)
                            nc.scalar.activation(escr[:nl, :], s[:nl, :], AF.Exp,
                                                 scale=inv_t, bias=nshift[:nl, 0:1])
                            _tree_reduce_sum(
                                nc, lambda lo, hi: escr[:nl, lo:hi], K,
                                z[:nl, 0:1])
                            d = small.tile([P, 1], F32, tag="d")
                            mscr = scrp.tile([P, K], BF16, tag="mscr",
                                             name=f"mscr{j}")
                            nc.vector.tensor_mul(mscr[:nl, :], s[:nl, :],
                                                 t_sb[lt][:nl, :])
                            _tree_reduce_sum(
                                nc, lambda lo, hi: mscr[:nl, lo:hi], K,
                                d[:nl, 0:1])
                            lse = small.tile([P, 1], F32, tag="lse")
                            nc.scalar.activation(lse[:nl, 0:1], z[:nl, 0:1], AF.Ln)
                            pp = small.tile([P, 1], F32, tag="pp")
                            nc.vector.scalar_tensor_tensor(
                                pp[:nl, 0:1], in0=d[:nl, 0:1], scalar=-inv_t,
                                in1=lse[:nl, 0:1], op0=ALU.mult, op1=ALU.add)
                            nc.vector.tensor_mul(res[:nl, j:j + 1], pp[:nl, 0:1],
                                                 mask_sb[:nl, j:j + 1])

            acc = small.tile([P, 1], F32, tag="acc")
            nc.vector.reduce_sum(acc[:, 0:1], res[:, :], axis=AX.X)
            nc.sync.dma_start(out_ext[0:P, 0:1], acc[:, 0:1])
            nc.sync.dma_start(out_ext[P:2 * P, 0:1], rsq[:, 0:1])

    nc.compile()
    return nc


_NC_CACHE = {}


def _get_nc(key, builder):
    if key not in _NC_CACHE:
        _NC_CACHE[key] = builder()
    return _NC_CACHE[key]


def prepare_inputs(student_Q, teacher_Q, recon, label, prototype,
                   patches_labels, epoch, B_loc, L, K, C, R):
    """Host-side prep: dtype conversion, sharding, per-core in_maps."""
    KSH = K // C
    n_lt = _ceil_div(L, P)
    n_t = B_loc * n_lt
    epoch = int(np.asarray(epoch))

    student_Q = np.asarray(student_Q, dtype=np.float32)
    teacher_Q = np.asarray(teacher_Q, dtype=np.float32)
    recon = np.asarray(recon, dtype=np.float32)
    label = np.asarray(label, dtype=np.float32)
    prototype = np.asarray(prototype, dtype=np.float32)
    patches_labels = np.asarray(patches_labels)

    bf = ml_dtypes.bfloat16
    s_bf = student_Q.astype(bf)
    t_bf = teacher_Q.astype(bf)
    r_bf = recon.reshape(C, P, R).astype(bf)
    lb_bf = label.reshape(C, P, R).astype(bf)

    if epoch == 0:
        pscale, iscale = 0.0, 1.0 / (C * B_loc * SK_EPS)
    else:
        pscale = PROTO_MOMENTUM / SK_EPS
        iscale = (1.0 - PROTO_MOMENTUM) / (C * B_loc * SK_EPS)

    proto_s = (prototype[0] * pscale).astype(np.float32)        # [L, K]
    mask_full = (patches_labels == 0).astype(np.float32)        # [B, L]

    cfg_arr = np.full((P, 1), iscale, dtype=np.float32)

    in_maps = []
    for c in range(C):
        b0 = c * B_loc
        m = np.zeros((P, n_t), dtype=np.float32)
        for b in range(B_loc):
            for lt in range(n_lt):
                nl = min(P, L - lt * P)
                m[:nl, b * n_lt + lt] = mask_full[b0 + b, lt * P:lt * P + nl]
        in_maps.append({
            "student": np.ascontiguousarray(s_bf[b0:b0 + B_loc]),
            "teacher": np.ascontiguousarray(t_bf[b0:b0 + B_loc]),
            "recon": np.ascontiguousarray(r_bf[c]),
            "label": np.ascontiguousarray(lb_bf[c]),
            "proto": np.ascontiguousarray(proto_s[:, c * KSH:(c + 1) * KSH]),
            "maskp": m,
            "cfg": cfg_arr,
        })
    mask_cnt = float(mask_full.sum())
    return in_maps, mask_cnt


def finalize(results, mask_cnt, recon_size, B_loc=8, L=196):
    cst_num = 0.0
    rsq = 0.0
    for r in results:
        o = np.asarray(r["out"], dtype=np.float64).reshape(-1)
        cst_num += o[:P].sum()
        rsq += o[P:].sum()
    loss = cst_num / mask_cnt + LSE_SHIFT + rsq / recon_size
    return np.asarray(loss, dtype=np.float32).reshape(())


def kernel(student_Q, teacher_Q, recon, label, prototype, patches_labels,
           epoch, _trace=False):
    B, L, K = 64, 196, 8192
    C = N_CORES
    B_loc = B // C
    R = B_loc * 3 * 224 * 224 // P
    nc = _get_nc(("full",), lambda: build_nc(B_loc, L, K, C, R))
    in_maps, mask_cnt = prepare_inputs(
        student_Q, teacher_Q, recon, label, prototype, patches_labels, epoch,
        B_loc, L, K, C, R)
    res = run_bass_kernel_spmd(nc, in_maps, list(range(C)), trace=_trace)
    out = finalize(res.results, mask_cnt, float(np.asarray(recon).size), B_loc, L)
    if _trace:
        return out, res
    return out



# revision 2
# speedup vs baseline: 1.1303x; 1.1303x over previous
"""Trainium2 Bass kernel v3 for nn_AFiReLoss (sinkhorn CE + recon MSE).

v3 over v2:
  * teacher in fp8e4 with DoubleRow pair-matmuls (2 batches contracted per
    instruction at 2x rate) -> teacher phase ~3x faster, half the DMA.
  * the CE dot term uses ReduceScatter of the masked student sum ms
    (bf16) + a tiny local shard dot against t, instead of AllGathering t
    -> the collective overlaps the sinkhorn tail instead of serializing.
  * sinkhorn row-step reciprocals use reciprocal_approx_fast (1-lane
    [1,512] ops were 3.3us each with the exact iterative divide).

Distribution (8 cores, SPMD): teacher K-sharded (no collective for the
batch sum); sinkhorn K-sharded with one [196] f32 AllReduce per iter;
student masked rows gathered host-side, l-sorted, round-robin across
cores; LSE via fused exp+accum on ScalarE; ms[l,k] via PE selection
matmuls; recon MSE on DVE with fused square-reduce (stt accum).
"""

import numpy as np
import ml_dtypes

import concourse.bass as bass
import concourse.mybir as mybir
from concourse import tile, bacc
from concourse.bass_utils import run_bass_kernel_spmd
from concourse.masks import make_identity

F32 = mybir.dt.float32
BF16 = mybir.dt.bfloat16
FP8 = mybir.dt.float8e4
AX = mybir.AxisListType
ALU = mybir.AluOpType
AF = mybir.ActivationFunctionType
DR = mybir.MatmulPerfMode.DoubleRow

P = 128
N_CORES = 8
L = 196
K = 8192
KSH = K // N_CORES           # 1024
B = 64
STUDENT_TEMP = 0.1
PROTO_MOMENTUM = 0.75
SK_EPS = 0.05
SK_ITERS = 3
LSE_SHIFT = 25.0
R = (B // N_CORES) * 3 * 224 * 224 // P      # 9408 recon elems/partition
LT_SIZES = [128, L - 128]                     # 2 l-tiles: 128 + 68

TEACH_GRP = 8                 # batches per teacher DMA group
RCH = 2352                    # recon chunk columns
ECH = 4096                    # exp chunk columns
MSCH = 512                    # ms psum chunk columns (1 PSUM bank)


def build_nc(n_gt, halves, debug_taps=(), stage=5):
    """stage: 1=teacher 2=+sinkhorn 3=+student-lse 4=+recon 5=full."""
    C = N_CORES
    n_rows = n_gt * P

    nc = bacc.Bacc("TRN2", target_bir_lowering=False, debug=False,
                   num_devices=C)

    sg = nc.declare_dram_parameter("sg", [n_rows, K], BF16, isOutput=False)
    teach = nc.declare_dram_parameter("teach", [B, L, KSH], FP8, isOutput=False)
    recon = nc.declare_dram_parameter("recon", [P, R], BF16, isOutput=False)
    label = nc.declare_dram_parameter("label", [P, R], BF16, isOutput=False)
    proto = nc.declare_dram_parameter("proto", [L, KSH], F32, isOutput=False)
    selm = nc.declare_dram_parameter("selm", [P, n_gt * 2 * P], BF16, isOutput=False)
    wvec = nc.declare_dram_parameter("wvec", [P, n_gt], F32, isOutput=False)
    cfg = nc.declare_dram_parameter("cfg", [P, 1], F32, isOutput=False)
    out_ext = nc.declare_dram_parameter("out", [P, 4], F32, isOutput=True)
    dbg = {}
    if "bsum" in debug_taps:
        dbg["bsum"] = nc.declare_dram_parameter("dbg_bsum", [L, KSH], F32,
                                                isOutput=True)
    if "t" in debug_taps:
        dbg["t"] = nc.declare_dram_parameter("dbg_t", [L, KSH], F32,
                                             isOutput=True)

    groups = [list(range(C))]
    n_tg = B // TEACH_GRP

    with tile.TileContext(nc) as tc:
        with (
            tc.tile_pool(name="dram", bufs=1, space="DRAM") as dram,
            tc.tile_pool(name="consts", bufs=1) as consts,
            tc.tile_pool(name="small", bufs=2) as small,
            tc.tile_pool(name="sgp", bufs=max(1, n_gt)) as sgp,
            tc.tile_pool(name="scrp", bufs=2) as scrp,
        ):
            # ---- DRAM scratch ----
            ms_tiled = dram.tile([C, L, KSH], BF16, tag="ms_tiled")
            ms_shard = dram.tile([L, KSH], BF16, tag="ms_shard")
            col_io = [
                (dram.tile([L, 1], F32, tag=f"col_in{i}", name=f"col_in{i}"),
                 dram.tile([L, 1], F32, tag=f"col_out{i}", name=f"col_out{i}"))
                for i in range(SK_ITERS)
            ]

            # ---- constants ----
            id2 = consts.tile([P, 2 * P], FP8, tag="id2")
            make_identity(nc, id2[:, 0:P])
            make_identity(nc, id2[:, P:2 * P])
            ones_col = consts.tile([P, 1], F32, tag="ones_col")
            nc.gpsimd.memset(ones_col[:, :], 1.0)
            ones_row = consts.tile([1, P], F32, tag="ones_row")
            nc.gpsimd.memset(ones_row[:, :], 1.0)
            nshift = consts.tile([P, 1], F32, tag="nshift")
            nc.gpsimd.memset(nshift[:, :], -LSE_SHIFT)
            cfg_sb = consts.tile([P, 1], F32, tag="cfg_sb")
            nc.sync.dma_start(cfg_sb[:, :], cfg[:, :])
            w_sb = consts.tile([P, n_gt], F32, tag="w_sb")
            nc.sync.dma_start(w_sb[:, :], wvec[:, :])
            sel_sb = consts.tile([P, n_gt * 2 * P], BF16, tag="sel_sb")
            nc.sync.dma_start(sel_sb[:, :], selm[:, :])

            # =========================================================
            # Teacher K-shard batch-sum: fp8 DoubleRow pair matmuls
            # =========================================================
            q_sb = []
            n_ch = KSH // MSCH
            id3 = id2[:, :].rearrange("p (t m) -> p t m", t=2)
            with (
                tc.tile_pool(name="tpool", bufs=3) as tpool,
                tc.tile_pool(name="bsp", bufs=2, space="PSUM") as bsp,
            ):
                for lt, nl in enumerate(LT_SIZES):
                    l0 = lt * P
                    psums = [bsp.tile([P, MSCH], F32, tag="bs",
                                      name=f"bs{lt}_{i}") for i in range(n_ch)]
                    for g in range(n_tg):
                        tt = tpool.tile([P, TEACH_GRP * KSH], FP8, tag="tt",
                                        name=f"tt{lt}")
                        nc.sync.dma_start(
                            tt[:nl, :].rearrange("l (b k) -> l b k", b=TEACH_GRP),
                            teach[g * TEACH_GRP:(g + 1) * TEACH_GRP,
                                  l0:l0 + nl, :].rearrange("b l k -> l b k"))
                        t3 = tt[:nl, :].rearrange("l (b k) -> l b k", b=TEACH_GRP)
                        for pr_i in range(TEACH_GRP // 2):
                            for ch in range(n_ch):
                                nc.tensor.matmul(
                                    psums[ch][:nl, :],
                                    id3[:nl, :, :nl],
                                    t3[:, 2 * pr_i:2 * pr_i + 2,
                                       ch * MSCH:(ch + 1) * MSCH],
                                    start=(g == 0 and pr_i == 0),
                                    stop=(g == n_tg - 1 and
                                          pr_i == TEACH_GRP // 2 - 1),
                                    perf_mode=DR)
                    qtile = consts.tile([P, KSH], F32, tag=f"q{lt}",
                                        name=f"q{lt}")
                    for ch in range(n_ch):
                        nc.scalar.copy(qtile[:nl, ch * MSCH:(ch + 1) * MSCH],
                                       psums[ch][:nl, :])
                    q_sb.append(qtile)
                if "bsum" in dbg:
                    for lt, nl in enumerate(LT_SIZES):
                        nc.sync.dma_start(dbg["bsum"][lt * P:lt * P + nl, :],
                                          q_sb[lt][:nl, :])

            # =========================================================
            # Sinkhorn (K-sharded, normal [l, k] layout). Constant factors
            # cancel through later normalizations; the last col-step uses
            # exactly 1/colsum (which folds the final t = q*K scaling).
            # ms selection-matmul chunks are emitted interleaved into the
            # AllReduce wait windows so the PE queue never starves.
            # =========================================================
            tb = []
            if stage >= 2:
                with tc.high_priority():
                    with tc.tile_pool(name="prp", bufs=1) as prp:
                        for lt, nl in enumerate(LT_SIZES):
                            l0 = lt * P
                            pr = prp.tile([P, KSH], F32, tag="pr",
                                          name=f"pr{lt}")
                            nc.sync.dma_start(pr[:nl, :], proto[l0:l0 + nl, :])
                            nc.vector.scalar_tensor_tensor(
                                q_sb[lt][:nl, :], in0=q_sb[lt][:nl, :],
                                scalar=cfg_sb[:nl, 0:1], in1=pr[:nl, :],
                                op0=ALU.mult, op1=ALU.add)
                            nc.scalar.activation(q_sb[lt][:nl, :],
                                                 q_sb[lt][:nl, :], AF.Exp)

            # =========================================================
            # Student: gathered masked rows; fused exp+accum LSE
            # =========================================================
            res = None
            sgt = []
            if stage >= 3:
                n_ech = K // ECH
                zp = consts.tile([P, n_gt * n_ech], F32, tag="zp")
                for t in range(n_gt):
                    s = sgp.tile([P, K], BF16, tag="sg", name=f"sg{t}")
                    nc.sync.dma_start(s[:, :], sg[t * P:(t + 1) * P, :])
                    sgt.append(s)
                    for j in range(n_ech):
                        escr = scrp.tile([P, ECH], BF16, tag="escr",
                                         name=f"escr{t}_{j}")
                        nc.scalar.activation(
                            escr[:, :], s[:, j * ECH:(j + 1) * ECH], AF.Exp,
                            scale=1.0 / STUDENT_TEMP, bias=nshift[:, 0:1],
                            accum_out=zp[:, t * n_ech + j:t * n_ech + j + 1])

                zz = small.tile([P, n_gt], F32, tag="zz")
                nc.vector.reduce_sum(
                    zz[:, :], zp[:, :].rearrange("p (t j) -> p t j", t=n_gt),
                    axis=AX.X)
                lse = small.tile([P, n_gt], F32, tag="lse")
                nc.scalar.activation(lse[:, :], zz[:, :], AF.Ln)
                wl = small.tile([P, n_gt], F32, tag="wl")
                nc.vector.tensor_mul(wl[:, :], lse[:, :], w_sb[:, :])
                res = small.tile([P, 1], F32, tag="res")
                nc.vector.reduce_sum(res[:, 0:1], wl[:, :], axis=AX.X)

            # =========================================================
            # Interleaved sinkhorn iterations + ms chunks
            # =========================================================
            n_sch = KSH // 512
            n_msch = K // MSCH
            dot_tab = None
            if stage >= 2:
                with (
                    tc.tile_pool(name="skp_r", bufs=1, space="PSUM") as skp_r,
                    tc.tile_pool(name="skp_b", bufs=2, space="PSUM") as skp_b,
                    tc.tile_pool(name="msp", bufs=3, space="PSUM") as msp,
                    tc.tile_pool(name="mst", bufs=3) as mst,
                ):
                    def emit_ms_chunks(cis):
                        for ci in cis:
                            c0 = ci * MSCH
                            cc = (ci * MSCH) // KSH
                            koff = (ci * MSCH) % KSH
                            for h in (0, 1):
                                nlh = LT_SIZES[h]
                                lh0 = h * P
                                mb = mst.tile([P, MSCH], BF16, tag="mb",
                                              name=f"mb{ci}_{h}")
                                todo = [t for t in range(n_gt)
                                        if h in halves[t]]
                                if todo:
                                    pm = msp.tile([P, MSCH], F32, tag="pm",
                                                  name=f"pm{ci}_{h}")
                                    for i, t in enumerate(todo):
                                        nc.tensor.matmul(
                                            pm[:nlh, :],
                                            sel_sb[:, (t * 2 + h) * P:
                                                   (t * 2 + h) * P + nlh],
                                            sgt[t][:, c0:c0 + MSCH],
                                            start=(i == 0),
                                            stop=(i == len(todo) - 1))
                                    nc.scalar.copy(mb[:nlh, :], pm[:nlh, :])
                                else:
                                    nc.vector.memset(mb[:nlh, :], 0.0)
                                nc.sync.dma_start(
                                    ms_tiled[cc, lh0:lh0 + nlh,
                                             koff:koff + MSCH],
                                    mb[:nlh, :])

                    for it in range(SK_ITERS):
                        # --- row factors rb[k] = 1/sum-over-l on PSUM ---
                        rr = small.tile([1, KSH], F32, tag="rr",
                                        name=f"rr{it}")
                        ps_bs = []
                        for ch in range(n_sch):
                            ps_r = skp_r.tile([1, 512], F32, tag="ps_r",
                                              name=f"ps_r{it}_{ch}")
                            for lt, nl in enumerate(LT_SIZES):
                                nc.tensor.matmul(
                                    ps_r[0:1, :], ones_col[:nl, 0:1],
                                    q_sb[lt][:nl, ch * 512:(ch + 1) * 512],
                                    start=(lt == 0), stop=(lt == 1))
                            nc.vector.reciprocal_approx_fast(
                                rr[0:1, ch * 512:(ch + 1) * 512],
                                ps_r[0:1, :])
                        for ch in range(n_sch):
                            ps_b = skp_b.tile([P, 512], F32, tag="ps_b",
                                              name=f"ps_b{it}_{ch}")
                            nc.tensor.matmul(
                                ps_b[:, :], ones_row[0:1, :],
                                rr[0:1, ch * 512:(ch + 1) * 512],
                                start=True, stop=True)
                            ps_bs.append(ps_b)

                        # --- col sums of (q * rb) via fused stt-accum ---
                        col_in, col_out = col_io[it]
                        for lt, nl in enumerate(LT_SIZES):
                            l0 = lt * P
                            ca = small.tile([P, 1], F32, tag="colpa",
                                            name=f"colpa{it}_{lt}")
                            cbm = small.tile([P, 1], F32, tag="colpb",
                                             name=f"colpb{it}_{lt}")
                            dsc = scrp.tile([P, 512], F32, tag="dsc",
                                            name=f"dsc{it}_{lt}a")
                            nc.vector.scalar_tensor_tensor(
                                dsc[:nl, :], in0=q_sb[lt][:nl, 0:512],
                                scalar=1.0, in1=ps_bs[0][:nl, :],
                                op0=ALU.mult, op1=ALU.mult,
                                accum_out=ca[:nl, 0:1])
                            dsc2 = scrp.tile([P, 512], F32, tag="dsc",
                                             name=f"dsc{it}_{lt}b")
                            nc.vector.scalar_tensor_tensor(
                                dsc2[:nl, :], in0=q_sb[lt][:nl, 512:1024],
                                scalar=ca[:nl, 0:1], in1=ps_bs[1][:nl, :],
                                op0=ALU.mult, op1=ALU.mult,
                                accum_out=cbm[:nl, 0:1])
                            nc.sync.dma_start(col_in[l0:l0 + nl, 0:1],
                                              cbm[:nl, 0:1])
                        nc.gpsimd.collective_compute(
                            "AllReduce", ALU.add, replica_groups=groups,
                            ins=[col_in.opt()], outs=[col_out.opt()])

                        # --- ms chunks into this AR's wait window ---
                        if stage >= 5:
                            if it == 1:
                                emit_ms_chunks(range(0, 6))
                            elif it == 2:
                                emit_ms_chunks(range(6, 12))

                        # --- apply q = (q * crec[l]) * rb[k] in one pass ---
                        for lt, nl in enumerate(LT_SIZES):
                            l0 = lt * P
                            csb = small.tile([P, 1], F32, tag="csb",
                                             name=f"csb{it}_{lt}")
                            nc.sync.dma_start(csb[:nl, 0:1],
                                              col_out[l0:l0 + nl, 0:1])
                            crec = small.tile([P, 1], F32, tag="crec",
                                              name=f"crec{it}_{lt}")
                            nc.vector.reciprocal(crec[:nl, 0:1],
                                                 csb[:nl, 0:1])
                            for ch in range(n_sch):
                                nc.vector.scalar_tensor_tensor(
                                    q_sb[lt][:nl, ch * 512:(ch + 1) * 512],
                                    in0=q_sb[lt][:nl,
                                                 ch * 512:(ch + 1) * 512],
                                    scalar=crec[:nl, 0:1],
                                    in1=ps_bs[ch][:nl, :],
                                    op0=ALU.mult, op1=ALU.mult)

                    # --- final t (bf16), kept in SBUF for the shard dot ---
                    for lt, nl in enumerate(LT_SIZES):
                        tbt = consts.tile([P, KSH], BF16, tag=f"tb{lt}",
                                          name=f"tb{lt}")
                        nc.vector.tensor_copy(tbt[:nl, :], q_sb[lt][:nl, :])
                        tb.append(tbt)
                        if "t" in dbg:
                            nc.sync.dma_start(dbg["t"][lt * P:lt * P + nl, :],
                                              q_sb[lt][:nl, :])
                    if stage >= 5:
                        emit_ms_chunks(range(12, n_msch))

                if stage >= 5:
                    nc.gpsimd.collective_compute(
                        "ReduceScatter", ALU.add, replica_groups=groups,
                        ins=[ms_tiled.opt()], outs=[ms_shard.opt()])

            # =========================================================
            # Recon MSE (DVE sub + fused square-reduce into columns)
            # =========================================================
            racc_tab = None
            if stage >= 4:
                n_rch = (R + RCH - 1) // RCH
                racc_tab = consts.tile([P, n_rch], F32, tag="racc_tab")
                nc.vector.memset(racc_tab[:, :], 0.0)
                with tc.tile_pool(name="rec", bufs=2) as rec:
                    for rc in range(n_rch):
                        r0 = rc * RCH
                        rw = min(RCH, R - r0)
                        r_sb = rec.tile([P, RCH], BF16, tag="r_sb",
                                        name=f"r_sb{rc}")
                        l_sb = rec.tile([P, RCH], BF16, tag="l_sb",
                                        name=f"l_sb{rc}")
                        nc.sync.dma_start(r_sb[:, :rw], recon[:, r0:r0 + rw])
                        nc.sync.dma_start(l_sb[:, :rw], label[:, r0:r0 + rw])
                        nc.vector.tensor_sub(r_sb[:, :rw], r_sb[:, :rw],
                                             l_sb[:, :rw])
                        trash = rec.tile([P, RCH], BF16, tag="trash",
                                         name=f"trash{rc}")
                        nc.vector.scalar_tensor_tensor(
                            trash[:, :rw], in0=r_sb[:, :rw], scalar=1.0,
                            in1=r_sb[:, :rw], op0=ALU.mult, op1=ALU.mult,
                            accum_out=racc_tab[:, rc:rc + 1])

            # =========================================================
            # Local shard dot: sum over (l, k-shard) of t * ms
            # =========================================================
            if stage >= 5:
                dot_tab = consts.tile([P, 2], F32, tag="dot_tab")
                nc.vector.memset(dot_tab[:, :], 0.0)
                with tc.tile_pool(name="msl", bufs=1) as msl:
                    for lt, nl in enumerate(LT_SIZES):
                        l0 = lt * P
                        msh = msl.tile([P, KSH], BF16, tag="msh",
                                       name=f"msh{lt}")
                        nc.sync.dma_start(msh[:nl, :],
                                          ms_shard[l0:l0 + nl, :])
                        dtr = msl.tile([P, KSH], BF16, tag="dtr",
                                       name=f"dtr{lt}")
                        nc.vector.scalar_tensor_tensor(
                            dtr[:nl, :], in0=msh[:nl, :], scalar=1.0,
                            in1=tb[lt][:nl, :], op0=ALU.mult, op1=ALU.mult,
                            accum_out=dot_tab[:nl, lt:lt + 1])

            # =========================================================
            # Assemble outputs: [res, rsq, dot, 0]
            # =========================================================
            outs = small.tile([P, 4], F32, tag="outs")
            nc.vector.memset(outs[:, :], 0.0)
            if res is not None:
                nc.vector.tensor_copy(outs[:, 0:1], res[:, 0:1])
            if racc_tab is not None:
                nc.vector.reduce_sum(outs[:, 1:2], racc_tab[:, :], axis=AX.X)
            if dot_tab is not None:
                nc.vector.reduce_sum(outs[:, 2:3], dot_tab[:, :], axis=AX.X)
            nc.sync.dma_start(out_ext[:, :], outs[:, :])

    nc.compile()
    return nc


_NC_CACHE = {}


def _get_nc(key, builder):
    if key not in _NC_CACHE:
        _NC_CACHE[key] = builder()
    return _NC_CACHE[key]


def prepare_inputs(student_Q, teacher_Q, recon, label, prototype,
                   patches_labels, epoch):
    C = N_CORES
    epoch = int(np.asarray(epoch))
    bf = ml_dtypes.bfloat16
    f8 = ml_dtypes.float8_e4m3fn

    student_Q = np.asarray(student_Q, dtype=np.float32)
    teacher_Q = np.asarray(teacher_Q, dtype=np.float32)
    recon = np.asarray(recon, dtype=np.float32)
    label = np.asarray(label, dtype=np.float32)
    prototype = np.asarray(prototype, dtype=np.float32)
    patches_labels = np.asarray(patches_labels)

    mask_flat = (patches_labels.reshape(-1) == 0)
    idx = np.nonzero(mask_flat)[0].astype(np.int64)
    lvals = (idx % L).astype(np.int64)
    order = np.argsort(lvals, kind="stable")
    idx = idx[order]
    M = idx.shape[0]
    per_core = -(-M // C)
    n_gt = -(-per_core // P)
    cap = n_gt * P

    s2d = student_Q.reshape(B * L, K)
    sg_list, w_list, sel_list = [], [], []
    halves_union = [set() for _ in range(n_gt)]
    for c in range(C):
        cidx = idx[c::C]
        nr = cidx.shape[0]
        sg_c = np.zeros((cap, K), dtype=bf)
        sg_c[:nr] = s2d[cidx].astype(bf)
        w_c = np.zeros(cap, dtype=np.float32)
        w_c[:nr] = 1.0
        l_c = np.zeros(cap, dtype=np.int64)
        l_c[:nr] = cidx % L
        sel_c = np.zeros((n_gt, 2, P, P), dtype=bf)
        tt = np.arange(cap) // P
        pp = np.arange(cap) % P
        hh = (l_c >= P).astype(np.int64)
        ll = l_c - hh * P
        valid = np.arange(cap) < nr
        sel_c[tt[valid], hh[valid], pp[valid], ll[valid]] = 1.0
        for t in range(n_gt):
            for h in (0, 1):
                if sel_c[t, h].any():
                    halves_union[t].add(h)
        sg_list.append(sg_c)
        w_list.append(np.ascontiguousarray(w_c.reshape(n_gt, P).T))
        sel_list.append(np.ascontiguousarray(
            sel_c.transpose(2, 0, 1, 3).reshape(P, n_gt * 2 * P)))

    halves = tuple(tuple(sorted(s)) for s in halves_union)

    t_f8 = teacher_Q.astype(f8)

    if epoch == 0:
        pscale, iscale = 0.0, 1.0 / (B * SK_EPS)
    else:
        pscale = PROTO_MOMENTUM / SK_EPS
        iscale = (1.0 - PROTO_MOMENTUM) / (B * SK_EPS)
    proto_s = prototype[0] * pscale
    cfg_arr = np.full((P, 1), iscale, dtype=np.float32)

    r_bf = recon.reshape(C, P, R).astype(bf)
    lb_bf = label.reshape(C, P, R).astype(bf)

    in_maps = []
    for c in range(C):
        in_maps.append({
            "sg": sg_list[c],
            "teach": np.ascontiguousarray(t_f8[:, :, c * KSH:(c + 1) * KSH]),
            "recon": np.ascontiguousarray(r_bf[c]),
            "label": np.ascontiguousarray(lb_bf[c]),
            "proto": np.ascontiguousarray(
                proto_s[:, c * KSH:(c + 1) * KSH].astype(np.float32)),
            "selm": sel_list[c],
            "wvec": w_list[c],
            "cfg": cfg_arr,
        })
    return in_maps, float(M), n_gt, halves


def finalize(results, mask_cnt, recon_size):
    res = rsq = dot = 0.0
    for r in results:
        o = np.asarray(r["out"], dtype=np.float64)
        res += o[:, 0].sum()
        rsq += o[:, 1].sum()
        dot += o[:, 2].sum()
    cst = (res + LSE_SHIFT * mask_cnt - dot / STUDENT_TEMP) / mask_cnt
    loss = cst + rsq / recon_size
    return np.asarray(loss, dtype=np.float32).reshape(())


def kernel(student_Q, teacher_Q, recon, label, prototype, patches_labels,
           epoch, _trace=False, _debug_taps=(), _stage=5):
    in_maps, mask_cnt, n_gt, halves = prepare_inputs(
        student_Q, teacher_Q, recon, label, prototype, patches_labels, epoch)
    nc = _get_nc((n_gt, halves, tuple(_debug_taps), _stage),
                 lambda: build_nc(n_gt, halves, tuple(_debug_taps), _stage))
    res = run_bass_kernel_spmd(nc, in_maps, list(range(N_CORES)),
                               trace=_trace)
    out = finalize(res.results, mask_cnt, float(np.asarray(recon).size))
    if _trace or _debug_taps:
        return out, res
    return out


# revision 3
# speedup vs baseline: 1.1797x; 1.0438x over previous
"""Trainium2 Bass kernel v3 for nn_AFiReLoss (sinkhorn CE + recon MSE).

v3 over v2:
  * teacher in fp8e4 with DoubleRow pair-matmuls (2 batches contracted per
    instruction at 2x rate) -> teacher phase ~3x faster, half the DMA.
  * the CE dot term uses ReduceScatter of the masked student sum ms
    (bf16) + a tiny local shard dot against t, instead of AllGathering t
    -> the collective overlaps the sinkhorn tail instead of serializing.
  * sinkhorn row-step reciprocals use reciprocal_approx_fast (1-lane
    [1,512] ops were 3.3us each with the exact iterative divide).

Distribution (8 cores, SPMD): teacher K-sharded (no collective for the
batch sum); sinkhorn K-sharded with one [196] f32 AllReduce per iter;
student masked rows gathered host-side, l-sorted, round-robin across
cores; LSE via fused exp+accum on ScalarE; ms[l,k] via PE selection
matmuls; recon MSE on DVE with fused square-reduce (stt accum).
"""

import numpy as np
import ml_dtypes

import concourse.bass as bass
import concourse.mybir as mybir
from concourse import tile, bacc
from concourse.bass_utils import run_bass_kernel_spmd
from concourse.masks import make_identity

F32 = mybir.dt.float32
BF16 = mybir.dt.bfloat16
FP8 = mybir.dt.float8e4
AX = mybir.AxisListType
ALU = mybir.AluOpType
AF = mybir.ActivationFunctionType
DR = mybir.MatmulPerfMode.DoubleRow

P = 128
N_CORES = 8
L = 196
K = 8192
KSH = K // N_CORES           # 1024
B = 64
STUDENT_TEMP = 0.1
PROTO_MOMENTUM = 0.75
SK_EPS = 0.05
SK_ITERS = 3
LSE_SHIFT = 25.0
R = (B // N_CORES) * 3 * 224 * 224 // P      # 9408 recon elems/partition
LT_SIZES = [128, L - 128]                     # 2 l-tiles: 128 + 68

TEACH_GRP = 8                 # batches per teacher DMA group
RCH = 2352                    # recon chunk columns
ECH = 4096                    # exp chunk columns
MSCH = 512                    # ms psum chunk columns (1 PSUM bank)


def build_nc(n_gt, halves, debug_taps=(), stage=5):
    """stage: 1=teacher 2=+sinkhorn 3=+student-lse 4=+recon 5=full."""
    C = N_CORES
    n_rows = n_gt * P

    nc = bacc.Bacc("TRN2", target_bir_lowering=False, debug=False,
                   num_devices=C)

    sg = nc.declare_dram_parameter("sg", [n_rows, K], BF16, isOutput=False)
    teach = nc.declare_dram_parameter("teach", [B, L, KSH], FP8, isOutput=False)
    recon = nc.declare_dram_parameter("recon", [P, R], BF16, isOutput=False)
    label = nc.declare_dram_parameter("label", [P, R], BF16, isOutput=False)
    proto = nc.declare_dram_parameter("proto", [L, KSH], F32, isOutput=False)
    selm = nc.declare_dram_parameter("selm", [P, n_gt * 2 * P], BF16, isOutput=False)
    wvec = nc.declare_dram_parameter("wvec", [P, n_gt], F32, isOutput=False)
    cfg = nc.declare_dram_parameter("cfg", [P, 1], F32, isOutput=False)
    out_ext = nc.declare_dram_parameter("out", [P, 8], F32, isOutput=True)
    dbg = {}
    if "bsum" in debug_taps:
        dbg["bsum"] = nc.declare_dram_parameter("dbg_bsum", [L, KSH], F32,
                                                isOutput=True)
    if "t" in debug_taps:
        dbg["t"] = nc.declare_dram_parameter("dbg_t", [L, KSH], F32,
                                             isOutput=True)

    groups = [list(range(C))]
    n_tg = B // TEACH_GRP

    with tile.TileContext(nc) as tc:
        with (
            tc.tile_pool(name="dram", bufs=1, space="DRAM") as dram,
            tc.tile_pool(name="consts", bufs=1) as consts,
            tc.tile_pool(name="small", bufs=2) as small,
            tc.tile_pool(name="sgp", bufs=max(1, n_gt)) as sgp,
            tc.tile_pool(name="scrp", bufs=2) as scrp,
        ):
            # ---- DRAM scratch ----
            ms_tiled = dram.tile([C, L, KSH], BF16, tag="ms_tiled")
            ms_shard = dram.tile([L, KSH], BF16, tag="ms_shard")
            col_io = [
                (dram.tile([L, 1], F32, tag=f"col_in{i}", name=f"col_in{i}"),
                 dram.tile([L, 1], F32, tag=f"col_out{i}", name=f"col_out{i}"))
                for i in range(SK_ITERS)
            ]

            # ---- constants ----
            id2 = consts.tile([P, 2 * P], FP8, tag="id2")
            make_identity(nc, id2[:, 0:P])
            make_identity(nc, id2[:, P:2 * P])
            ones_col = consts.tile([P, 1], F32, tag="ones_col")
            nc.gpsimd.memset(ones_col[:, :], 1.0)
            ones_row = consts.tile([1, P], F32, tag="ones_row")
            nc.gpsimd.memset(ones_row[:, :], 1.0)
            nshift = consts.tile([P, 1], F32, tag="nshift")
            nc.gpsimd.memset(nshift[:, :], -LSE_SHIFT)
            cfg_sb = consts.tile([P, 1], F32, tag="cfg_sb")
            nc.sync.dma_start(cfg_sb[:, :], cfg[:, :])
            w_sb = consts.tile([P, n_gt], F32, tag="w_sb")
            nc.sync.dma_start(w_sb[:, :], wvec[:, :])
            sel_sb = consts.tile([P, n_gt * 2 * P], BF16, tag="sel_sb")
            nc.sync.dma_start(sel_sb[:, :], selm[:, :])

            # =========================================================
            # Teacher K-shard batch-sum: fp8 DoubleRow pair matmuls
            # =========================================================
            q_sb = []
            n_ch = KSH // MSCH
            id3 = id2[:, :].rearrange("p (t m) -> p t m", t=2)
            with (
                tc.tile_pool(name="tpool", bufs=3) as tpool,
                tc.tile_pool(name="bsp", bufs=2, space="PSUM") as bsp,
            ):
                for lt, nl in enumerate(LT_SIZES):
                    l0 = lt * P
                    psums = [bsp.tile([P, MSCH], F32, tag="bs",
                                      name=f"bs{lt}_{i}") for i in range(n_ch)]
                    for g in range(n_tg):
                        tt = tpool.tile([P, TEACH_GRP * KSH], FP8, tag="tt",
                                        name=f"tt{lt}")
                        nc.sync.dma_start(
                            tt[:nl, :].rearrange("l (b k) -> l b k", b=TEACH_GRP),
                            teach[g * TEACH_GRP:(g + 1) * TEACH_GRP,
                                  l0:l0 + nl, :].rearrange("b l k -> l b k"))
                        t3 = tt[:nl, :].rearrange("l (b k) -> l b k", b=TEACH_GRP)
                        for pr_i in range(TEACH_GRP // 2):
                            for ch in range(n_ch):
                                nc.tensor.matmul(
                                    psums[ch][:nl, :],
                                    id3[:nl, :, :nl],
                                    t3[:, 2 * pr_i:2 * pr_i + 2,
                                       ch * MSCH:(ch + 1) * MSCH],
                                    start=(g == 0 and pr_i == 0),
                                    stop=(g == n_tg - 1 and
                                          pr_i == TEACH_GRP // 2 - 1),
                                    perf_mode=DR)
                    qtile = consts.tile([P, KSH], F32, tag=f"q{lt}",
                                        name=f"q{lt}")
                    for ch in range(n_ch):
                        nc.scalar.copy(qtile[:nl, ch * MSCH:(ch + 1) * MSCH],
                                       psums[ch][:nl, :])
                    q_sb.append(qtile)
                if "bsum" in dbg:
                    for lt, nl in enumerate(LT_SIZES):
                        nc.sync.dma_start(dbg["bsum"][lt * P:lt * P + nl, :],
                                          q_sb[lt][:nl, :])

            # =========================================================
            # Sinkhorn (K-sharded, normal [l, k] layout). Constant factors
            # cancel through later normalizations; the last col-step uses
            # exactly 1/colsum (which folds the final t = q*K scaling).
            # ms selection-matmul chunks are emitted interleaved into the
            # AllReduce wait windows so the PE queue never starves.
            # =========================================================
            tb = []
            if stage >= 2:
                with tc.high_priority():
                    with tc.tile_pool(name="prp", bufs=1) as prp:
                        for lt, nl in enumerate(LT_SIZES):
                            l0 = lt * P
                            pr = prp.tile([P, KSH], F32, tag="pr",
                                          name=f"pr{lt}")
                            nc.sync.dma_start(pr[:nl, :], proto[l0:l0 + nl, :])
                            nc.vector.scalar_tensor_tensor(
                                q_sb[lt][:nl, :], in0=q_sb[lt][:nl, :],
                                scalar=cfg_sb[:nl, 0:1], in1=pr[:nl, :],
                                op0=ALU.mult, op1=ALU.add)
                            nc.scalar.activation(q_sb[lt][:nl, :],
                                                 q_sb[lt][:nl, :], AF.Exp)

            # =========================================================
            # Student: gathered masked rows; fused exp+accum LSE
            # =========================================================
            res = None
            sgt = []
            if stage >= 3:
                n_ech = K // ECH
                zp = consts.tile([P, n_gt * n_ech], F32, tag="zp")
                for t in range(n_gt):
                    s = sgp.tile([P, K], BF16, tag="sg", name=f"sg{t}")
                    nc.sync.dma_start(s[:, :], sg[t * P:(t + 1) * P, :])
                    sgt.append(s)
                    for j in range(n_ech):
                        escr = scrp.tile([P, ECH], BF16, tag="escr",
                                         name=f"escr{t}_{j}")
                        nc.scalar.activation(
                            escr[:, :], s[:, j * ECH:(j + 1) * ECH], AF.Exp,
                            scale=1.0 / STUDENT_TEMP, bias=nshift[:, 0:1],
                            accum_out=zp[:, t * n_ech + j:t * n_ech + j + 1])

                zz = small.tile([P, n_gt], F32, tag="zz")
                nc.vector.reduce_sum(
                    zz[:, :], zp[:, :].rearrange("p (t j) -> p t j", t=n_gt),
                    axis=AX.X)
                lse = small.tile([P, n_gt], F32, tag="lse")
                nc.scalar.activation(lse[:, :], zz[:, :], AF.Ln)
                wl = small.tile([P, n_gt], F32, tag="wl")
                nc.vector.tensor_mul(wl[:, :], lse[:, :], w_sb[:, :])
                res = small.tile([P, 1], F32, tag="res")
                nc.vector.reduce_sum(res[:, 0:1], wl[:, :], axis=AX.X)

            # =========================================================
            # Interleaved sinkhorn iterations + ms chunks
            # =========================================================
            n_sch = KSH // 512
            n_msch = K // MSCH
            dot_tab = None
            if stage >= 2:
                with (
                    tc.tile_pool(name="skp_r", bufs=1, space="PSUM") as skp_r,
                    tc.tile_pool(name="skp_b", bufs=2, space="PSUM") as skp_b,
                    tc.tile_pool(name="msp", bufs=3, space="PSUM") as msp,
                    tc.tile_pool(name="mst", bufs=3) as mst,
                ):
                    def emit_ms_chunks(cis):
                        for ci in cis:
                            c0 = ci * MSCH
                            cc = (ci * MSCH) // KSH
                            koff = (ci * MSCH) % KSH
                            for h in (0, 1):
                                nlh = LT_SIZES[h]
                                lh0 = h * P
                                mb = mst.tile([P, MSCH], BF16, tag="mb",
                                              name=f"mb{ci}_{h}")
                                todo = [t for t in range(n_gt)
                                        if h in halves[t]]
                                if todo:
                                    pm = msp.tile([P, MSCH], F32, tag="pm",
                                                  name=f"pm{ci}_{h}")
                                    for i, t in enumerate(todo):
                                        nc.tensor.matmul(
                                            pm[:nlh, :],
                                            sel_sb[:, (t * 2 + h) * P:
                                                   (t * 2 + h) * P + nlh],
                                            sgt[t][:, c0:c0 + MSCH],
                                            start=(i == 0),
                                            stop=(i == len(todo) - 1))
                                    nc.scalar.copy(mb[:nlh, :], pm[:nlh, :])
                                else:
                                    nc.vector.memset(mb[:nlh, :], 0.0)
                                nc.sync.dma_start(
                                    ms_tiled[cc, lh0:lh0 + nlh,
                                             koff:koff + MSCH],
                                    mb[:nlh, :])

                    for it in range(SK_ITERS - 1):
                        # --- row factors rb[k] = 1/sum-over-l on PSUM ---
                        rr = small.tile([1, KSH], F32, tag="rr",
                                        name=f"rr{it}")
                        ps_bs = []
                        for ch in range(n_sch):
                            ps_r = skp_r.tile([1, 512], F32, tag="ps_r",
                                              name=f"ps_r{it}_{ch}")
                            for lt, nl in enumerate(LT_SIZES):
                                nc.tensor.matmul(
                                    ps_r[0:1, :], ones_col[:nl, 0:1],
                                    q_sb[lt][:nl, ch * 512:(ch + 1) * 512],
                                    start=(lt == 0), stop=(lt == 1))
                            nc.vector.reciprocal_approx_fast(
                                rr[0:1, ch * 512:(ch + 1) * 512],
                                ps_r[0:1, :])
                        for ch in range(n_sch):
                            ps_b = skp_b.tile([P, 512], F32, tag="ps_b",
                                              name=f"ps_b{it}_{ch}")
                            nc.tensor.matmul(
                                ps_b[:, :], ones_row[0:1, :],
                                rr[0:1, ch * 512:(ch + 1) * 512],
                                start=True, stop=True)
                            ps_bs.append(ps_b)

                        # --- col sums of (q * rb) via fused stt-accum ---
                        col_in, col_out = col_io[it]
                        for lt, nl in enumerate(LT_SIZES):
                            l0 = lt * P
                            ca = small.tile([P, 1], F32, tag="colpa",
                                            name=f"colpa{it}_{lt}")
                            cbm = small.tile([P, 1], F32, tag="colpb",
                                             name=f"colpb{it}_{lt}")
                            dsc = scrp.tile([P, 512], F32, tag="dsc",
                                            name=f"dsc{it}_{lt}a")
                            nc.vector.scalar_tensor_tensor(
                                dsc[:nl, :], in0=q_sb[lt][:nl, 0:512],
                                scalar=1.0, in1=ps_bs[0][:nl, :],
                                op0=ALU.mult, op1=ALU.mult,
                                accum_out=ca[:nl, 0:1])
                            dsc2 = scrp.tile([P, 512], F32, tag="dsc",
                                             name=f"dsc{it}_{lt}b")
                            nc.vector.scalar_tensor_tensor(
                                dsc2[:nl, :], in0=q_sb[lt][:nl, 512:1024],
                                scalar=ca[:nl, 0:1], in1=ps_bs[1][:nl, :],
                                op0=ALU.mult, op1=ALU.mult,
                                accum_out=cbm[:nl, 0:1])
                            nc.sync.dma_start(col_in[l0:l0 + nl, 0:1],
                                              cbm[:nl, 0:1])
                        nc.gpsimd.collective_compute(
                            "AllReduce", ALU.add, replica_groups=groups,
                            ins=[col_in.opt()], outs=[col_out.opt()])

                        # --- ms chunks into this AR's wait window ---
                        if stage >= 5:
                            if it == 0:
                                emit_ms_chunks(range(0, 12))
                            else:
                                emit_ms_chunks(range(12, n_msch))

                        # --- apply q = (q * crec[l]) * rb[k] in one pass ---
                        for lt, nl in enumerate(LT_SIZES):
                            l0 = lt * P
                            csb = small.tile([P, 1], F32, tag="csb",
                                             name=f"csb{it}_{lt}")
                            nc.sync.dma_start(csb[:nl, 0:1],
                                              col_out[l0:l0 + nl, 0:1])
                            crec = small.tile([P, 1], F32, tag="crec",
                                              name=f"crec{it}_{lt}")
                            nc.vector.reciprocal(crec[:nl, 0:1],
                                                 csb[:nl, 0:1])
                            for ch in range(n_sch):
                                nc.vector.scalar_tensor_tensor(
                                    q_sb[lt][:nl, ch * 512:(ch + 1) * 512],
                                    in0=q_sb[lt][:nl,
                                                 ch * 512:(ch + 1) * 512],
                                    scalar=crec[:nl, 0:1],
                                    in1=ps_bs[ch][:nl, :],
                                    op0=ALU.mult, op1=ALU.mult)

                    # --- last iteration: apply rb locally, AR the col
                    # sums early; the final 1/colsum is folded into the
                    # shard dot as a per-partition scale ---
                    it = SK_ITERS - 1
                    rr = small.tile([1, KSH], F32, tag="rr", name=f"rr{it}")
                    ps_bs = []
                    for ch in range(n_sch):
                        ps_r = skp_r.tile([1, 512], F32, tag="ps_r",
                                          name=f"ps_r{it}_{ch}")
                        for lt, nl in enumerate(LT_SIZES):
                            nc.tensor.matmul(
                                ps_r[0:1, :], ones_col[:nl, 0:1],
                                q_sb[lt][:nl, ch * 512:(ch + 1) * 512],
                                start=(lt == 0), stop=(lt == 1))
                        nc.vector.reciprocal_approx_fast(
                            rr[0:1, ch * 512:(ch + 1) * 512], ps_r[0:1, :])
                    for ch in range(n_sch):
                        ps_b = skp_b.tile([P, 512], F32, tag="ps_b",
                                          name=f"ps_b{it}_{ch}")
                        nc.tensor.matmul(
                            ps_b[:, :], ones_row[0:1, :],
                            rr[0:1, ch * 512:(ch + 1) * 512],
                            start=True, stop=True)
                        ps_bs.append(ps_b)
                    cs_f = []
                    for lt, nl in enumerate(LT_SIZES):
                        for ch in range(n_sch):
                            nc.vector.tensor_mul(
                                q_sb[lt][:nl, ch * 512:(ch + 1) * 512],
                                q_sb[lt][:nl, ch * 512:(ch + 1) * 512],
                                ps_bs[ch][:nl, :])
                        colp = small.tile([P, 1], F32, tag="colpf",
                                          name=f"colpf{lt}")
                        nc.vector.reduce_sum(colp[:nl, 0:1], q_sb[lt][:nl, :],
                                             axis=AX.X)
                        cs_f.append(colp)
                    if "t" in dbg:
                        for lt, nl in enumerate(LT_SIZES):
                            nc.sync.dma_start(dbg["t"][lt * P:lt * P + nl, :],
                                              q_sb[lt][:nl, :])
                    tb = cs_f

                if stage >= 5:
                    nc.gpsimd.collective_compute(
                        "ReduceScatter", ALU.add, replica_groups=groups,
                        ins=[ms_tiled.opt()], outs=[ms_shard.opt()])

            # =========================================================
            # Recon MSE (DVE sub + fused square-reduce into columns)
            # =========================================================
            racc_tab = None
            if stage >= 4:
                n_rch = (R + RCH - 1) // RCH
                racc_tab = consts.tile([P, n_rch], F32, tag="racc_tab")
                nc.vector.memset(racc_tab[:, :], 0.0)
                with tc.tile_pool(name="rec", bufs=2) as rec:
                    for rc in range(n_rch):
                        r0 = rc * RCH
                        rw = min(RCH, R - r0)
                        r_sb = rec.tile([P, RCH], BF16, tag="r_sb",
                                        name=f"r_sb{rc}")
                        l_sb = rec.tile([P, RCH], BF16, tag="l_sb",
                                        name=f"l_sb{rc}")
                        nc.sync.dma_start(r_sb[:, :rw], recon[:, r0:r0 + rw])
                        nc.sync.dma_start(l_sb[:, :rw], label[:, r0:r0 + rw])
                        nc.vector.tensor_sub(r_sb[:, :rw], r_sb[:, :rw],
                                             l_sb[:, :rw])
                        trash = rec.tile([P, RCH], BF16, tag="trash",
                                         name=f"trash{rc}")
                        nc.vector.scalar_tensor_tensor(
                            trash[:, :rw], in0=r_sb[:, :rw], scalar=1.0,
                            in1=r_sb[:, :rw], op0=ALU.mult, op1=ALU.mult,
                            accum_out=racc_tab[:, rc:rc + 1])

            # =========================================================
            # Local shard dot: sum over (l, k-shard) of t * ms
            # =========================================================
            if stage >= 5:
                dot_tab = consts.tile([P, 2], F32, tag="dot_tab")
                nc.vector.memset(dot_tab[:, :], 0.0)
                with tc.tile_pool(name="msl", bufs=1) as msl:
                    for lt, nl in enumerate(LT_SIZES):
                        l0 = lt * P
                        msh = msl.tile([P, KSH], BF16, tag="msh",
                                       name=f"msh{lt}")
                        nc.sync.dma_start(msh[:nl, :],
                                          ms_shard[l0:l0 + nl, :])
                        dtr = msl.tile([P, KSH], BF16, tag="dtr",
                                       name=f"dtr{lt}")
                        nc.vector.scalar_tensor_tensor(
                            dtr[:nl, :], in0=msh[:nl, :], scalar=1.0,
                            in1=q_sb[lt][:nl, :], op0=ALU.mult, op1=ALU.mult,
                            accum_out=dot_tab[:nl, lt:lt + 1])

            # =========================================================
            # Assemble outputs: [res, rsq, dot, 0]
            # =========================================================
            outs = small.tile([P, 8], F32, tag="outs")
            nc.vector.memset(outs[:, :], 0.0)
            if res is not None:
                nc.vector.tensor_copy(outs[:, 0:1], res[:, 0:1])
            if racc_tab is not None:
                nc.vector.reduce_sum(outs[:, 1:2], racc_tab[:, :], axis=AX.X)
            if dot_tab is not None:
                nc.vector.tensor_copy(outs[:, 2:4], dot_tab[:, :])
                for lt, nl in enumerate(LT_SIZES):
                    nc.vector.tensor_copy(outs[:nl, 4 + lt:5 + lt],
                                          tb[lt][:nl, 0:1])
            nc.sync.dma_start(out_ext[:, :], outs[:, :])

    nc.compile()
    return nc


_NC_CACHE = {}


def _get_nc(key, builder):
    if key not in _NC_CACHE:
        _NC_CACHE[key] = builder()
    return _NC_CACHE[key]


def prepare_inputs(student_Q, teacher_Q, recon, label, prototype,
                   patches_labels, epoch):
    C = N_CORES
    epoch = int(np.asarray(epoch))
    bf = ml_dtypes.bfloat16
    f8 = ml_dtypes.float8_e4m3fn

    student_Q = np.asarray(student_Q, dtype=np.float32)
    teacher_Q = np.asarray(teacher_Q, dtype=np.float32)
    recon = np.asarray(recon, dtype=np.float32)
    label = np.asarray(label, dtype=np.float32)
    prototype = np.asarray(prototype, dtype=np.float32)
    patches_labels = np.asarray(patches_labels)

    mask_flat = (patches_labels.reshape(-1) == 0)
    idx = np.nonzero(mask_flat)[0].astype(np.int64)
    lvals = (idx % L).astype(np.int64)
    order = np.argsort(lvals, kind="stable")
    idx = idx[order]
    M = idx.shape[0]
    per_core = -(-M // C)
    n_gt = -(-per_core // P)
    cap = n_gt * P

    s2d = student_Q.reshape(B * L, K)
    sg_list, w_list, sel_list = [], [], []
    halves_union = [set() for _ in range(n_gt)]
    for c in range(C):
        cidx = idx[c::C]
        nr = cidx.shape[0]
        sg_c = np.zeros((cap, K), dtype=bf)
        sg_c[:nr] = s2d[cidx].astype(bf)
        w_c = np.zeros(cap, dtype=np.float32)
        w_c[:nr] = 1.0
        l_c = np.zeros(cap, dtype=np.int64)
        l_c[:nr] = cidx % L
        sel_c = np.zeros((n_gt, 2, P, P), dtype=bf)
        tt = np.arange(cap) // P
        pp = np.arange(cap) % P
        hh = (l_c >= P).astype(np.int64)
        ll = l_c - hh * P
        valid = np.arange(cap) < nr
        sel_c[tt[valid], hh[valid], pp[valid], ll[valid]] = 1.0
        for t in range(n_gt):
            for h in (0, 1):
                if sel_c[t, h].any():
                    halves_union[t].add(h)
        sg_list.append(sg_c)
        w_list.append(np.ascontiguousarray(w_c.reshape(n_gt, P).T))
        sel_list.append(np.ascontiguousarray(
            sel_c.transpose(2, 0, 1, 3).reshape(P, n_gt * 2 * P)))

    halves = tuple(tuple(sorted(s)) for s in halves_union)

    t_f8 = teacher_Q.astype(f8)

    if epoch == 0:
        pscale, iscale = 0.0, 1.0 / (B * SK_EPS)
    else:
        pscale = PROTO_MOMENTUM / SK_EPS
        iscale = (1.0 - PROTO_MOMENTUM) / (B * SK_EPS)
    proto_s = prototype[0] * pscale
    cfg_arr = np.full((P, 1), iscale, dtype=np.float32)

    r_bf = recon.reshape(C, P, R).astype(bf)
    lb_bf = label.reshape(C, P, R).astype(bf)

    in_maps = []
    for c in range(C):
        in_maps.append({
            "sg": sg_list[c],
            "teach": np.ascontiguousarray(t_f8[:, :, c * KSH:(c + 1) * KSH]),
            "recon": np.ascontiguousarray(r_bf[c]),
            "label": np.ascontiguousarray(lb_bf[c]),
            "proto": np.ascontiguousarray(
                proto_s[:, c * KSH:(c + 1) * KSH].astype(np.float32)),
            "selm": sel_list[c],
            "wvec": w_list[c],
            "cfg": cfg_arr,
        })
    return in_maps, float(M), n_gt, halves


def finalize(results, mask_cnt, recon_size):
    res = rsq = 0.0
    dv = np.zeros((P, 2))
    cs = np.zeros((P, 2))
    for r in results:
        o = np.asarray(r["out"], dtype=np.float64)
        res += o[:, 0].sum()
        rsq += o[:, 1].sum()
        dv += o[:, 2:4]
        cs += o[:, 4:6]
    nl1 = LT_SIZES[1]
    with np.errstate(divide="ignore", invalid="ignore"):
        rat = np.where(cs > 0, dv / np.where(cs > 0, cs, 1.0), 0.0)
    dot = rat[:, 0].sum() + rat[:nl1, 1].sum()
    cst = (res + LSE_SHIFT * mask_cnt - dot / STUDENT_TEMP) / mask_cnt
    loss = cst + rsq / recon_size
    return np.asarray(loss, dtype=np.float32).reshape(())


def kernel(student_Q, teacher_Q, recon, label, prototype, patches_labels,
           epoch, _trace=False, _debug_taps=(), _stage=5):
    in_maps, mask_cnt, n_gt, halves = prepare_inputs(
        student_Q, teacher_Q, recon, label, prototype, patches_labels, epoch)
    nc = _get_nc((n_gt, halves, tuple(_debug_taps), _stage),
                 lambda: build_nc(n_gt, halves, tuple(_debug_taps), _stage))
    res = run_bass_kernel_spmd(nc, in_maps, list(range(N_CORES)),
                               trace=_trace)
    out = finalize(res.results, mask_cnt, float(np.asarray(recon).size))
    if _trace or _debug_taps:
        return out, res
    return out


# revision 4
# speedup vs baseline: 1.2375x; 1.0490x over previous
"""Trainium2 Bass kernel v4 for nn_AFiReLoss (sinkhorn CE + recon MSE).

v3 over v2:
  * teacher in fp8e4 with DoubleRow pair-matmuls (2 batches contracted per
    instruction at 2x rate) -> teacher phase ~3x faster, half the DMA.
  * the CE dot term uses ReduceScatter of the masked student sum ms
    (bf16) + a tiny local shard dot against t, instead of AllGathering t
    -> the collective overlaps the sinkhorn tail instead of serializing.
  * sinkhorn row-step reciprocals use reciprocal_approx_fast (1-lane
    [1,512] ops were 3.3us each with the exact iterative divide).

Distribution (8 cores, SPMD): teacher K-sharded (no collective for the
batch sum); sinkhorn K-sharded with one [196] f32 AllReduce per iter;
student masked rows gathered host-side, l-sorted, round-robin across
cores; LSE via fused exp+accum on ScalarE; ms[l,k] via PE selection
matmuls; recon MSE on DVE with fused square-reduce (stt accum).
"""

import numpy as np
import ml_dtypes

import concourse.bass as bass
import concourse.mybir as mybir
from concourse import tile, bacc
from concourse.bass_utils import run_bass_kernel_spmd
from concourse.masks import make_identity

F32 = mybir.dt.float32
BF16 = mybir.dt.bfloat16
FP8 = mybir.dt.float8e4
AX = mybir.AxisListType
ALU = mybir.AluOpType
AF = mybir.ActivationFunctionType
DR = mybir.MatmulPerfMode.DoubleRow

P = 128
N_CORES = 8
L = 196
K = 8192
KSH = K // N_CORES           # 1024
B = 64
STUDENT_TEMP = 0.1
PROTO_MOMENTUM = 0.75
SK_EPS = 0.05
SK_ITERS = 3
LSE_SHIFT = 25.0
R = (B // N_CORES) * 3 * 224 * 224 // P      # 9408 recon elems/partition
LT_SIZES = [128, L - 128]                     # 2 l-tiles: 128 + 68

TEACH_GRP = 8                 # batches per teacher DMA group
RCH = 2352                    # recon chunk columns
ECH = 4096                    # exp chunk columns
MSCH = 512                    # ms psum chunk columns (1 PSUM bank)


def build_nc(n_gt, halves, debug_taps=(), stage=5):
    """stage: 1=teacher 2=+sinkhorn 3=+student-lse 4=+recon 5=full."""
    C = N_CORES
    n_rows = n_gt * P

    nc = bacc.Bacc("TRN2", target_bir_lowering=False, debug=False,
                   num_devices=C)

    sg = nc.declare_dram_parameter("sg", [n_rows, K], BF16, isOutput=False)
    teach = nc.declare_dram_parameter("teach", [B, L, KSH], FP8, isOutput=False)
    recon = nc.declare_dram_parameter("recon", [P, R], BF16, isOutput=False)
    label = nc.declare_dram_parameter("label", [P, R], BF16, isOutput=False)
    proto = nc.declare_dram_parameter("proto", [L, KSH], F32, isOutput=False)
    selm = nc.declare_dram_parameter("selm", [P, n_gt * 2 * P], BF16, isOutput=False)
    wvec = nc.declare_dram_parameter("wvec", [P, n_gt], F32, isOutput=False)
    cfg = nc.declare_dram_parameter("cfg", [P, 1], F32, isOutput=False)
    out_ext = nc.declare_dram_parameter("out", [P, 8], F32, isOutput=True)
    dbg = {}
    if "bsum" in debug_taps:
        dbg["bsum"] = nc.declare_dram_parameter("dbg_bsum", [L, KSH], F32,
                                                isOutput=True)
    if "t" in debug_taps:
        dbg["t"] = nc.declare_dram_parameter("dbg_t", [L, KSH], F32,
                                             isOutput=True)

    groups = [list(range(C))]
    n_tg = B // TEACH_GRP

    with tile.TileContext(nc) as tc:
        with (
            tc.tile_pool(name="dram", bufs=1, space="DRAM") as dram,
            tc.tile_pool(name="consts", bufs=1) as consts,
            tc.tile_pool(name="small", bufs=2) as small,
            tc.tile_pool(name="sgp", bufs=max(1, n_gt)) as sgp,
            tc.tile_pool(name="scrp", bufs=2) as scrp,
        ):
            # ---- DRAM scratch ----
            ms_tiled = dram.tile([C, L, KSH], BF16, tag="ms_tiled")
            ms_shard = dram.tile([L, KSH], BF16, tag="ms_shard")
            col_io = [
                (dram.tile([L, 1], F32, tag=f"col_in{i}", name=f"col_in{i}"),
                 dram.tile([L, 1], F32, tag=f"col_out{i}", name=f"col_out{i}"))
                for i in range(SK_ITERS)
            ]

            # ---- constants ----
            id2 = consts.tile([P, 2 * P], FP8, tag="id2")
            make_identity(nc, id2[:, 0:P])
            make_identity(nc, id2[:, P:2 * P])
            ones_col = consts.tile([P, 1], F32, tag="ones_col")
            nc.gpsimd.memset(ones_col[:, :], 1.0)
            ones_row = consts.tile([1, P], F32, tag="ones_row")
            nc.gpsimd.memset(ones_row[:, :], 1.0)
            nshift = consts.tile([P, 1], F32, tag="nshift")
            nc.gpsimd.memset(nshift[:, :], -LSE_SHIFT)
            cfg_sb = consts.tile([P, 1], F32, tag="cfg_sb")
            nc.sync.dma_start(cfg_sb[:, :], cfg[:, :])
            w_sb = consts.tile([P, n_gt], F32, tag="w_sb")
            nc.sync.dma_start(w_sb[:, :], wvec[:, :])
            sel_sb = consts.tile([P, n_gt * 2 * P], BF16, tag="sel_sb")
            nc.sync.dma_start(sel_sb[:, :], selm[:, :])

            # =========================================================
            # Teacher K-shard batch-sum: fp8 DoubleRow pair matmuls
            # =========================================================
            q_sb = []
            n_ch = KSH // MSCH
            id3 = id2[:, :].rearrange("p (t m) -> p t m", t=2)
            with (
                tc.tile_pool(name="tpool", bufs=3) as tpool,
                tc.tile_pool(name="bsp", bufs=2, space="PSUM") as bsp,
            ):
                for lt, nl in enumerate(LT_SIZES):
                    l0 = lt * P
                    psums = [bsp.tile([P, MSCH], F32, tag="bs",
                                      name=f"bs{lt}_{i}") for i in range(n_ch)]
                    for g in range(n_tg):
                        tt = tpool.tile([P, TEACH_GRP * KSH], FP8, tag="tt",
                                        name=f"tt{lt}")
                        nc.sync.dma_start(
                            tt[:nl, :].rearrange("l (b k) -> l b k", b=TEACH_GRP),
                            teach[g * TEACH_GRP:(g + 1) * TEACH_GRP,
                                  l0:l0 + nl, :].rearrange("b l k -> l b k"))
                        t3 = tt[:nl, :].rearrange("l (b k) -> l b k", b=TEACH_GRP)
                        for pr_i in range(TEACH_GRP // 2):
                            for ch in range(n_ch):
                                nc.tensor.matmul(
                                    psums[ch][:nl, :],
                                    id3[:nl, :, :nl],
                                    t3[:, 2 * pr_i:2 * pr_i + 2,
                                       ch * MSCH:(ch + 1) * MSCH],
                                    start=(g == 0 and pr_i == 0),
                                    stop=(g == n_tg - 1 and
                                          pr_i == TEACH_GRP // 2 - 1),
                                    perf_mode=DR)
                    qtile = consts.tile([P, KSH], F32, tag=f"q{lt}",
                                        name=f"q{lt}")
                    for ch in range(n_ch):
                        nc.scalar.copy(qtile[:nl, ch * MSCH:(ch + 1) * MSCH],
                                       psums[ch][:nl, :])
                    q_sb.append(qtile)
                if "bsum" in dbg:
                    for lt, nl in enumerate(LT_SIZES):
                        nc.sync.dma_start(dbg["bsum"][lt * P:lt * P + nl, :],
                                          q_sb[lt][:nl, :])

            # =========================================================
            # Sinkhorn (K-sharded, normal [l, k] layout). Constant factors
            # cancel through later normalizations; the last col-step uses
            # exactly 1/colsum (which folds the final t = q*K scaling).
            # ms selection-matmul chunks are emitted interleaved into the
            # AllReduce wait windows so the PE queue never starves.
            # =========================================================
            tb = []
            if stage >= 2:
                with tc.high_priority():
                    with tc.tile_pool(name="prp", bufs=1) as prp:
                        for lt, nl in enumerate(LT_SIZES):
                            l0 = lt * P
                            pr = prp.tile([P, KSH], F32, tag="pr",
                                          name=f"pr{lt}")
                            nc.sync.dma_start(pr[:nl, :], proto[l0:l0 + nl, :])
                            nc.vector.scalar_tensor_tensor(
                                q_sb[lt][:nl, :], in0=q_sb[lt][:nl, :],
                                scalar=cfg_sb[:nl, 0:1], in1=pr[:nl, :],
                                op0=ALU.mult, op1=ALU.add)
                            nc.scalar.activation(q_sb[lt][:nl, :],
                                                 q_sb[lt][:nl, :], AF.Exp)

            # =========================================================
            # Student: gathered masked rows; fused exp+accum LSE
            # =========================================================
            res = None
            sgt = []
            if stage >= 3:
                n_ech = K // ECH
                zp = consts.tile([P, n_gt * n_ech], F32, tag="zp")
                for t in range(n_gt):
                    s = sgp.tile([P, K], BF16, tag="sg", name=f"sg{t}")
                    nc.sync.dma_start(s[:, :], sg[t * P:(t + 1) * P, :])
                    sgt.append(s)
                    for j in range(n_ech):
                        escr = scrp.tile([P, ECH], BF16, tag="escr",
                                         name=f"escr{t}_{j}")
                        nc.scalar.activation(
                            escr[:, :], s[:, j * ECH:(j + 1) * ECH], AF.Exp,
                            scale=1.0 / STUDENT_TEMP, bias=nshift[:, 0:1],
                            accum_out=zp[:, t * n_ech + j:t * n_ech + j + 1])

                zz = small.tile([P, n_gt], F32, tag="zz")
                nc.vector.reduce_sum(
                    zz[:, :], zp[:, :].rearrange("p (t j) -> p t j", t=n_gt),
                    axis=AX.X)
                lse = small.tile([P, n_gt], F32, tag="lse")
                nc.scalar.activation(lse[:, :], zz[:, :], AF.Ln)
                wl = small.tile([P, n_gt], F32, tag="wl")
                nc.vector.tensor_mul(wl[:, :], lse[:, :], w_sb[:, :])
                res = small.tile([P, 1], F32, tag="res")
                nc.vector.reduce_sum(res[:, 0:1], wl[:, :], axis=AX.X)

            # =========================================================
            # Interleaved sinkhorn iterations + ms chunks
            # =========================================================
            n_sch = KSH // 512
            n_msch = K // MSCH
            dot_tab = None
            if stage >= 2:
                with (
                    tc.tile_pool(name="skp_r", bufs=1, space="PSUM") as skp_r,
                    tc.tile_pool(name="skp_b", bufs=2, space="PSUM") as skp_b,
                    tc.tile_pool(name="msp", bufs=3, space="PSUM") as msp,
                    tc.tile_pool(name="mst", bufs=3) as mst,
                ):
                    def emit_ms_chunks(cis):
                        for ci in cis:
                            c0 = ci * MSCH
                            cc = (ci * MSCH) // KSH
                            koff = (ci * MSCH) % KSH
                            for h in (0, 1):
                                nlh = LT_SIZES[h]
                                lh0 = h * P
                                mb = mst.tile([P, MSCH], BF16, tag="mb",
                                              name=f"mb{ci}_{h}")
                                todo = [t for t in range(n_gt)
                                        if h in halves[t]]
                                if todo:
                                    pm = msp.tile([P, MSCH], F32, tag="pm",
                                                  name=f"pm{ci}_{h}")
                                    for i, t in enumerate(todo):
                                        nc.tensor.matmul(
                                            pm[:nlh, :],
                                            sel_sb[:, (t * 2 + h) * P:
                                                   (t * 2 + h) * P + nlh],
                                            sgt[t][:, c0:c0 + MSCH],
                                            start=(i == 0),
                                            stop=(i == len(todo) - 1))
                                    nc.scalar.copy(mb[:nlh, :], pm[:nlh, :])
                                else:
                                    nc.vector.memset(mb[:nlh, :], 0.0)
                                nc.sync.dma_start(
                                    ms_tiled[cc, lh0:lh0 + nlh,
                                             koff:koff + MSCH],
                                    mb[:nlh, :])

                    for it in range(SK_ITERS - 1):
                        # --- row factors rb[k] = 1/sum-over-l on PSUM ---
                        rr = small.tile([1, KSH], F32, tag="rr",
                                        name=f"rr{it}")
                        ps_bs = []
                        for ch in range(n_sch):
                            ps_r = skp_r.tile([1, 512], F32, tag="ps_r",
                                              name=f"ps_r{it}_{ch}")
                            for lt, nl in enumerate(LT_SIZES):
                                nc.tensor.matmul(
                                    ps_r[0:1, :], ones_col[:nl, 0:1],
                                    q_sb[lt][:nl, ch * 512:(ch + 1) * 512],
                                    start=(lt == 0), stop=(lt == 1))
                            nc.vector.reciprocal_approx_fast(
                                rr[0:1, ch * 512:(ch + 1) * 512],
                                ps_r[0:1, :])
                        for ch in range(n_sch):
                            ps_b = skp_b.tile([P, 512], F32, tag="ps_b",
                                              name=f"ps_b{it}_{ch}")
                            nc.tensor.matmul(
                                ps_b[:, :], ones_row[0:1, :],
                                rr[0:1, ch * 512:(ch + 1) * 512],
                                start=True, stop=True)
                            ps_bs.append(ps_b)

                        # --- col sums of (q * rb) via fused stt-accum ---
                        col_in, col_out = col_io[it]
                        for lt, nl in enumerate(LT_SIZES):
                            l0 = lt * P
                            ca = small.tile([P, 1], F32, tag="colpa",
                                            name=f"colpa{it}_{lt}")
                            cbm = small.tile([P, 1], F32, tag="colpb",
                                             name=f"colpb{it}_{lt}")
                            dsc = scrp.tile([P, 512], F32, tag="dsc",
                                            name=f"dsc{it}_{lt}a")
                            nc.vector.scalar_tensor_tensor(
                                dsc[:nl, :], in0=q_sb[lt][:nl, 0:512],
                                scalar=1.0, in1=ps_bs[0][:nl, :],
                                op0=ALU.mult, op1=ALU.mult,
                                accum_out=ca[:nl, 0:1])
                            dsc2 = scrp.tile([P, 512], F32, tag="dsc",
                                             name=f"dsc{it}_{lt}b")
                            nc.vector.scalar_tensor_tensor(
                                dsc2[:nl, :], in0=q_sb[lt][:nl, 512:1024],
                                scalar=ca[:nl, 0:1], in1=ps_bs[1][:nl, :],
                                op0=ALU.mult, op1=ALU.mult,
                                accum_out=cbm[:nl, 0:1])
                            nc.sync.dma_start(col_in[l0:l0 + nl, 0:1],
                                              cbm[:nl, 0:1])
                        nc.gpsimd.collective_compute(
                            "AllReduce", ALU.add, replica_groups=groups,
                            ins=[col_in.opt()], outs=[col_out.opt()])

                        # --- ms chunks into this AR's wait window ---
                        if stage >= 5:
                            if it == 0:
                                emit_ms_chunks(range(0, 12))
                            else:
                                emit_ms_chunks(range(12, n_msch))

                        # --- apply q = (q * crec[l]) * rb[k] in one pass ---
                        for lt, nl in enumerate(LT_SIZES):
                            l0 = lt * P
                            csb = small.tile([P, 1], F32, tag="csb",
                                             name=f"csb{it}_{lt}")
                            nc.sync.dma_start(csb[:nl, 0:1],
                                              col_out[l0:l0 + nl, 0:1])
                            crec = small.tile([P, 1], F32, tag="crec",
                                              name=f"crec{it}_{lt}")
                            nc.vector.reciprocal(crec[:nl, 0:1],
                                                 csb[:nl, 0:1])
                            for ch in range(n_sch):
                                nc.vector.scalar_tensor_tensor(
                                    q_sb[lt][:nl, ch * 512:(ch + 1) * 512],
                                    in0=q_sb[lt][:nl,
                                                 ch * 512:(ch + 1) * 512],
                                    scalar=crec[:nl, 0:1],
                                    in1=ps_bs[ch][:nl, :],
                                    op0=ALU.mult, op1=ALU.mult)

                    # --- last iteration: apply rb locally, AR the col
                    # sums early; the final 1/colsum is folded into the
                    # shard dot as a per-partition scale ---
                    it = SK_ITERS - 1
                    rr = small.tile([1, KSH], F32, tag="rr", name=f"rr{it}")
                    ps_bs = []
                    for ch in range(n_sch):
                        ps_r = skp_r.tile([1, 512], F32, tag="ps_r",
                                          name=f"ps_r{it}_{ch}")
                        for lt, nl in enumerate(LT_SIZES):
                            nc.tensor.matmul(
                                ps_r[0:1, :], ones_col[:nl, 0:1],
                                q_sb[lt][:nl, ch * 512:(ch + 1) * 512],
                                start=(lt == 0), stop=(lt == 1))
                        nc.vector.reciprocal_approx_fast(
                            rr[0:1, ch * 512:(ch + 1) * 512], ps_r[0:1, :])
                    for ch in range(n_sch):
                        ps_b = skp_b.tile([P, 512], F32, tag="ps_b",
                                          name=f"ps_b{it}_{ch}")
                        nc.tensor.matmul(
                            ps_b[:, :], ones_row[0:1, :],
                            rr[0:1, ch * 512:(ch + 1) * 512],
                            start=True, stop=True)
                        ps_bs.append(ps_b)
                    cs_f = []
                    for lt, nl in enumerate(LT_SIZES):
                        for ch in range(n_sch):
                            nc.vector.tensor_mul(
                                q_sb[lt][:nl, ch * 512:(ch + 1) * 512],
                                q_sb[lt][:nl, ch * 512:(ch + 1) * 512],
                                ps_bs[ch][:nl, :])
                        colp = small.tile([P, 1], F32, tag="colpf",
                                          name=f"colpf{lt}")
                        nc.vector.reduce_sum(colp[:nl, 0:1], q_sb[lt][:nl, :],
                                             axis=AX.X)
                        cs_f.append(colp)
                    if "t" in dbg:
                        for lt, nl in enumerate(LT_SIZES):
                            nc.sync.dma_start(dbg["t"][lt * P:lt * P + nl, :],
                                              q_sb[lt][:nl, :])
                    tb = cs_f

                if stage >= 5:
                    nc.gpsimd.collective_compute(
                        "ReduceScatter", ALU.add, replica_groups=groups,
                        ins=[ms_tiled.opt()], outs=[ms_shard.opt()])

            # =========================================================
            # Recon MSE (DVE sub + fused square-reduce into columns)
            # =========================================================
            racc_tab = None
            if stage >= 4:
                n_rch = (R + RCH - 1) // RCH
                racc_tab = consts.tile([P, n_rch], F32, tag="racc_tab")
                nc.vector.memset(racc_tab[:, :], 0.0)
                with tc.tile_pool(name="rec", bufs=2) as rec:
                    for rc in range(n_rch):
                        r0 = rc * RCH
                        rw = min(RCH, R - r0)
                        r_sb = rec.tile([P, RCH], BF16, tag="r_sb",
                                        name=f"r_sb{rc}")
                        l_sb = rec.tile([P, RCH], BF16, tag="l_sb",
                                        name=f"l_sb{rc}")
                        nc.sync.dma_start(r_sb[:, :rw], recon[:, r0:r0 + rw])
                        nc.sync.dma_start(l_sb[:, :rw], label[:, r0:r0 + rw])
                        nc.vector.tensor_sub(r_sb[:, :rw], r_sb[:, :rw],
                                             l_sb[:, :rw])
                        trash = rec.tile([P, RCH], BF16, tag="trash",
                                         name=f"trash{rc}")
                        nc.vector.scalar_tensor_tensor(
                            trash[:, :rw], in0=r_sb[:, :rw], scalar=1.0,
                            in1=r_sb[:, :rw], op0=ALU.mult, op1=ALU.mult,
                            accum_out=racc_tab[:, rc:rc + 1])

            # =========================================================
            # Local shard dot: sum over (l, k-shard) of t * ms
            # =========================================================
            if stage >= 5:
                dot_tab = consts.tile([P, 2], F32, tag="dot_tab")
                nc.vector.memset(dot_tab[:, :], 0.0)
                with tc.tile_pool(name="msl", bufs=1) as msl:
                    for lt, nl in enumerate(LT_SIZES):
                        l0 = lt * P
                        msh = msl.tile([P, KSH], BF16, tag="msh",
                                       name=f"msh{lt}")
                        nc.sync.dma_start(msh[:nl, :],
                                          ms_shard[l0:l0 + nl, :])
                        dtr = msl.tile([P, KSH], BF16, tag="dtr",
                                       name=f"dtr{lt}")
                        nc.vector.scalar_tensor_tensor(
                            dtr[:nl, :], in0=msh[:nl, :], scalar=1.0,
                            in1=q_sb[lt][:nl, :], op0=ALU.mult, op1=ALU.mult,
                            accum_out=dot_tab[:nl, lt:lt + 1])

            # =========================================================
            # Assemble outputs: [res, rsq, dot, 0]
            # =========================================================
            outs = small.tile([P, 8], F32, tag="outs")
            nc.vector.memset(outs[:, :], 0.0)
            if res is not None:
                nc.vector.tensor_copy(outs[:, 0:1], res[:, 0:1])
            if racc_tab is not None:
                nc.vector.reduce_sum(outs[:, 1:2], racc_tab[:, :], axis=AX.X)
            if dot_tab is not None:
                nc.vector.tensor_copy(outs[:, 2:4], dot_tab[:, :])
                for lt, nl in enumerate(LT_SIZES):
                    nc.vector.tensor_copy(outs[:nl, 4 + lt:5 + lt],
                                          tb[lt][:nl, 0:1])
            nc.sync.dma_start(out_ext[:, :], outs[:, :])

    nc.compile()
    return nc


_NC_CACHE = {}


def _get_nc(key, builder):
    if key not in _NC_CACHE:
        _NC_CACHE[key] = builder()
    return _NC_CACHE[key]


def prepare_inputs(student_Q, teacher_Q, recon, label, prototype,
                   patches_labels, epoch):
    C = N_CORES
    epoch = int(np.asarray(epoch))
    bf = ml_dtypes.bfloat16
    f8 = ml_dtypes.float8_e4m3fn

    student_Q = np.asarray(student_Q, dtype=np.float32)
    teacher_Q = np.asarray(teacher_Q, dtype=np.float32)
    recon = np.asarray(recon, dtype=np.float32)
    label = np.asarray(label, dtype=np.float32)
    prototype = np.asarray(prototype, dtype=np.float32)
    patches_labels = np.asarray(patches_labels)

    mask_flat = (patches_labels.reshape(-1) == 0)
    idx = np.nonzero(mask_flat)[0].astype(np.int64)
    lvals = (idx % L).astype(np.int64)
    order = np.argsort(lvals, kind="stable")
    idx = idx[order]
    M = idx.shape[0]
    per_core = -(-M // C)
    n_gt = -(-per_core // P)
    cap = n_gt * P

    s2d = student_Q.reshape(B * L, K)
    sg_list, w_list, sel_list = [], [], []
    halves_union = [set() for _ in range(n_gt)]
    for c in range(C):
        cidx = idx[c::C]
        nr = cidx.shape[0]
        sg_c = np.zeros((cap, K), dtype=bf)
        sg_c[:nr] = s2d[cidx].astype(bf)
        w_c = np.zeros(cap, dtype=np.float32)
        w_c[:nr] = 1.0
        l_c = np.zeros(cap, dtype=np.int64)
        l_c[:nr] = cidx % L
        sel_c = np.zeros((n_gt, 2, P, P), dtype=bf)
        tt = np.arange(cap) // P
        pp = np.arange(cap) % P
        hh = (l_c >= P).astype(np.int64)
        ll = l_c - hh * P
        valid = np.arange(cap) < nr
        sel_c[tt[valid], hh[valid], pp[valid], ll[valid]] = 1.0
        for t in range(n_gt):
            for h in (0, 1):
                if sel_c[t, h].any():
                    halves_union[t].add(h)
        sg_list.append(sg_c)
        w_list.append(np.ascontiguousarray(w_c.reshape(n_gt, P).T))
        sel_list.append(np.ascontiguousarray(
            sel_c.transpose(2, 0, 1, 3).reshape(P, n_gt * 2 * P)))

    halves = tuple(tuple(sorted(s)) for s in halves_union)

    t_f8 = teacher_Q.astype(f8)

    if epoch == 0:
        pscale, iscale = 0.0, 1.0 / (B * SK_EPS)
    else:
        pscale = PROTO_MOMENTUM / SK_EPS
        iscale = (1.0 - PROTO_MOMENTUM) / (B * SK_EPS)
    proto_s = prototype[0] * pscale
    cfg_arr = np.full((P, 1), iscale, dtype=np.float32)

    r_bf = recon.reshape(C, P, R).astype(bf)
    lb_bf = label.reshape(C, P, R).astype(bf)

    in_maps = []
    for c in range(C):
        in_maps.append({
            "sg": sg_list[c],
            "teach": np.ascontiguousarray(t_f8[:, :, c * KSH:(c + 1) * KSH]),
            "recon": np.ascontiguousarray(r_bf[c]),
            "label": np.ascontiguousarray(lb_bf[c]),
            "proto": np.ascontiguousarray(
                proto_s[:, c * KSH:(c + 1) * KSH].astype(np.float32)),
            "selm": sel_list[c],
            "wvec": w_list[c],
            "cfg": cfg_arr,
        })
    return in_maps, float(M), n_gt, halves


def finalize(results, mask_cnt, recon_size):
    res = rsq = 0.0
    dv = np.zeros((P, 2))
    cs = np.zeros((P, 2))
    for r in results:
        o = np.asarray(r["out"], dtype=np.float64)
        res += o[:, 0].sum()
        rsq += o[:, 1].sum()
        dv += o[:, 2:4]
        cs += o[:, 4:6]
    nl1 = LT_SIZES[1]
    with np.errstate(divide="ignore", invalid="ignore"):
        rat = np.where(cs > 0, dv / np.where(cs > 0, cs, 1.0), 0.0)
    dot = rat[:, 0].sum() + rat[:nl1, 1].sum()
    cst = (res + LSE_SHIFT * mask_cnt - dot / STUDENT_TEMP) / mask_cnt
    loss = cst + rsq / recon_size
    return np.asarray(loss, dtype=np.float32).reshape(())


def kernel(student_Q, teacher_Q, recon, label, prototype, patches_labels,
           epoch, _trace=False, _debug_taps=(), _stage=5):
    in_maps, mask_cnt, n_gt, halves = prepare_inputs(
        student_Q, teacher_Q, recon, label, prototype, patches_labels, epoch)
    nc = _get_nc((n_gt, halves, tuple(_debug_taps), _stage),
                 lambda: build_nc(n_gt, halves, tuple(_debug_taps), _stage))
    res = run_bass_kernel_spmd(nc, in_maps, list(range(N_CORES)),
                               trace=_trace)
    out = finalize(res.results, mask_cnt, float(np.asarray(recon).size))
    if _trace or _debug_taps:
        return out, res
    return out
